# revision 1
# baseline (speedup 1.0000x reference)
"""BiDirectionalTriangleAttention on 8 TRN2 NeuronCores (Bass/Tile SPMD).

Sharding: I (row) axis of x1/x_pair/mask split across 8 cores (128 rows each).
Per core:
  - triangle bias tri[h, i_loc, j] = einsum(x_pair, wb) computed from a
    host-pre-transposed x_pair shard ([i, c, j] layout, bf16) so the C
    contraction lands on SBUF partitions with zero on-chip transposes of the
    512MB tensor.  Bounced through DRAM to re-layout as [i_part, h, j].
  - mha_1 fully local (queries = local rows, keys = full x2n).
  - mha_2 computed flash-style as a *partial* softmax over the local key rows
    (keys/values = locally updated x1u shard), emitting per-head unnormalized
    o2 partials + exp-sums (ones-augmented V).  Host merges the 8 partials and
    applies the (tiny) gating + output projection + residual for x2u.
"""

import numpy as np
import ml_dtypes

import concourse.bass as bass
import concourse.bacc as bacc
import concourse.mybir as mybir
import concourse.tile as tile
from concourse.bass_utils import run_bass_kernel_spmd

F32 = mybir.dt.float32
BF16 = mybir.dt.bfloat16
BF = ml_dtypes.bfloat16
AX = mybir.AxisListType
ALU = mybir.AluOpType
ACTF = mybir.ActivationFunctionType

B, I, J, C, H, D = 1, 1024, 1024, 128, 8, 32
HD = H * D          # 256
NCORES = 8
IS = I // NCORES    # 128 rows per core
INF = 1e9
EPS = 1e-5
ISCALE = float(1.0 / np.sqrt(np.float32(D)))

IB = 4              # x_pair rows per DMA
SG = 8              # tri rows staged per scratch DMA


def _ln_tile(nc, pool, x, out_dtype, lnw_b, lnb_b, tag):
    """LayerNorm over the free (C) dim of x [P, C] -> new tile [P, C]."""
    P = x.shape[0]
    nsum = pool.tile([P, 1], F32, name=f"nsum_{tag}", tag=f"nsum_{tag}")
    nc.vector.tensor_reduce(nsum, x, axis=AX.X, op=ALU.add, negate=True)
    nc.vector.tensor_scalar_mul(nsum, nsum, 1.0 / C)          # -mu
    xc = pool.tile([P, C], F32, name=f"xc_{tag}", tag=f"xc_{tag}")
    nc.scalar.activation(xc, x, ACTF.Identity, bias=nsum, scale=1.0)  # x - mu
    sq = pool.tile([P, C], F32, name=f"sq_{tag}", tag=f"sq_{tag}")
    vs = pool.tile([P, 1], F32, name=f"vs_{tag}", tag=f"vs_{tag}")
    nc.scalar.activation(sq, xc, ACTF.Square, accum_out=vs)   # sum (x-mu)^2
    sd = pool.tile([P, 1], F32, name=f"sd_{tag}", tag=f"sd_{tag}")
    nc.scalar.activation(sd, vs, ACTF.Sqrt, bias=EPS, scale=1.0 / C)
    rstd = pool.tile([P, 1], F32, name=f"rstd_{tag}", tag=f"rstd_{tag}")
    nc.vector.reciprocal(rstd, sd)
    xn = pool.tile([P, C], F32, name=f"xn_{tag}", tag=f"xn_{tag}")
    nc.scalar.activation(xn, xc, ACTF.Copy, scale=rstd)
    nc.vector.tensor_mul(xn, xn, lnw_b)
    out = pool.tile([P, C], out_dtype, name=f"lnout_{tag}", tag=f"lnout_{tag}")
    nc.vector.tensor_add(out, xn, lnb_b)
    return out


def build_program():
    nc = bacc.Bacc("TRN2", target_bir_lowering=False, debug=False,
                   num_devices=NCORES)

    def din(name, shape, dt=F32):
        return nc.dram_tensor(name, shape, dt, kind="ExternalInput").ap()

    def dout(name, shape, dt=F32):
        return nc.dram_tensor(name, shape, dt, kind="ExternalOutput").ap()

    xpt = din("xpt", [IS, C, J], BF16)     # x_pair shard, [i, c, j] (host-transposed)
    x1s = din("x1s", [IS, C])
    x2d = din("x2d", [J, C])
    msk = din("msk", [IS, J])
    lnw = din("lnw", [128, C])             # row-tiled ln weight
    lnb = din("lnb", [128, C])
    wq1t = din("wq1t", [C, HD])
    wk1t = din("wk1t", [C, HD], BF16)
    wv1t = din("wv1t", [C, HD], BF16)
    wg1t = din("wg1t", [C, HD])
    wo1t = din("wo1t", [HD, C])
    bg1b = din("bg1b", [128, HD])
    bo1c = din("bo1c", [C, 1])
    wq2t = din("wq2t", [C, HD], BF16)
    wk2t = din("wk2t", [C, HD])
    wv2t = din("wv2t", [C, HD])
    wbt = din("wbt", [C, 4 * H], BF16)   # wb.T replicated 4x (col-pack)
    id32 = din("id32", [128, 128])
    idbf = din("idbf", [128, 128], BF16)

    x1u_o = dout("x1u_o", [IS, C])
    o2p_o = dout("o2p_o", [H, D + 1, J])

    with tile.TileContext(nc) as tc:
        cst = tc.alloc_tile_pool(name="cst", bufs=1)
        sb = tc.alloc_tile_pool(name="sb", bufs=1)
        wk = tc.alloc_tile_pool(name="wk", bufs=3)
        xpp = tc.alloc_tile_pool(name="xpp", bufs=4)
        stp = tc.alloc_tile_pool(name="stp", bufs=2)
        drp = tc.alloc_tile_pool(name="drp", bufs=1, space="DRAM")
        ptri = tc.alloc_tile_pool(name="ptri", bufs=2, space="PSUM")
        ptp = tc.alloc_tile_pool(name="ptp", bufs=2, space="PSUM")
        pmm = tc.alloc_tile_pool(name="pmm", bufs=2, space="PSUM")
        pacc = tc.alloc_tile_pool(name="pacc", bufs=2, space="PSUM")

        def load(pool, ap, name, dt=None, bufs=None):
            t = pool.tile(list(ap.shape), dt or ap.dtype, name=name, tag=name,
                          bufs=bufs)
            nc.sync.dma_start(t, ap)
            return t

        # const APs for float biases used by scalar.activation
        for cval in (0.0, EPS):
            cap = cst.tile([128, 1], F32, name=f"constap_{cval}",
                           tag=f"constap_{cval}")
            nc.vector.memset(cap, cval)
            nc.const_aps.aps[(F32, cval)] = cap

        # ---- constants / weights ----
        c_id32 = load(cst, id32, "c_id32")
        c_idbf = load(cst, idbf, "c_idbf")
        c_lnw = load(cst, lnw, "c_lnw")
        c_lnb = load(cst, lnb, "c_lnb")
        c_wq1t = load(cst, wq1t, "c_wq1t")
        c_wk1t = load(cst, wk1t, "c_wk1t")
        c_wv1t = load(cst, wv1t, "c_wv1t")
        c_wg1t = load(cst, wg1t, "c_wg1t")
        c_wo1t = cst.tile([128, 2, C], F32, name="c_wo1t", tag="c_wo1t")
        nc.sync.dma_start(c_wo1t, wo1t.rearrange("(t p) c -> p t c", p=128))
        c_bg1b = load(cst, bg1b, "c_bg1b")
        c_bo1c = load(cst, bo1c, "c_bo1c")
        c_wq2t = load(cst, wq2t, "c_wq2t")
        c_wk2t = load(cst, wk2t, "c_wk2t")
        c_wv2t = load(cst, wv2t, "c_wv2t")
        c_wbt = load(cst, wbt, "c_wbt")

        # ---- small inputs + LN ----
        t_x1 = load(sb, x1s, "t_x1")
        t_msk = load(sb, msk, "t_msk")

        # ---- triangle bias ----
        # 4 rows (i) per matmul group via tile_position col-packing: row i0+k's
        # [8, 512] output lands at psum partition base 32k, so the PSUM->SBUF
        # copy runs full-lane [128, 512].  Staged to DRAM scratch [g, p, j]
        # (rows 8-31 of each 32-group are garbage, skipped on reload).
        NG = IS // IB                       # 32 groups of 4 rows
        GS = 4                              # groups staged per scratch DMA
        tri_scr = drp.tile([NG, 128, J], BF16, name="tri_scr", tag="tri_scr")
        for g in range(NG):
            i0 = g * IB
            xt = xpp.tile([C, IB, J], BF16, name="xt", tag="xt")
            nc.sync.dma_start(xt, xpt[i0:i0 + IB].rearrange("i c j -> c i j"))
            if g % GS == 0:
                stg = stp.tile([128, GS, J], BF16, name="stg", tag="stg")
            for blk in range(2):
                ps = ptri.tile([128, 512], F32, name="ps_tri", tag="tri")
                for k in range(IB):
                    nc.tensor.matmul(ps[32 * k:32 * (k + 1), :], c_wbt,
                                     xt[:, k, blk * 512:(blk + 1) * 512],
                                     start=True, stop=True,
                                     tile_position=(0, 32 * k))
                dst = stg[:, g % GS, blk * 512:(blk + 1) * 512]
                if (g + blk) % 2 == 0:
                    nc.vector.tensor_copy(dst, ps)
                else:
                    nc.scalar.copy(dst, ps)
            if g % GS == GS - 1:
                nc.sync.dma_start(
                    tri_scr[g - GS + 1:g + 1].rearrange("g p j -> p g j"), stg)

        # ---- LN + projections (fill stream-tail gap) ----

        x1n = _ln_tile(nc, sb, t_x1, F32, c_lnw, c_lnb, "x1")
        tp = ptp.tile([128, 128], F32, name="tp_x1n", tag="tp")
        nc.tensor.transpose(tp, x1n, c_id32)
        x1nT = sb.tile([128, IS], F32, name="x1nT", tag="x1nT")
        nc.vector.tensor_copy(x1nT, tp)

        x2nT = sb.tile([128, J], BF16, name="x2nT", tag="x2nT")
        for jt in range(8):
            x2t = load(wk, x2d[jt * 128:(jt + 1) * 128, :], "x2t")
            x2n_jt = _ln_tile(nc, wk, x2t, BF16, c_lnw, c_lnb, "x2")
            tpb = ptp.tile([128, 128], BF16, name="tp_x2n", tag="tp")
            nc.tensor.transpose(tpb, x2n_jt, c_idbf)
            nc.vector.tensor_copy(x2nT[:, jt * 128:(jt + 1) * 128], tpb)

        # mask bias  mb = INF * (mask - 1)
        mb = sb.tile([IS, J], F32, name="mb", tag="mb")
        nc.scalar.activation(mb, t_msk, ACTF.Copy, bias=-INF, scale=INF)

        # ---- projections ----
        # q1T/k1T per head at partition base 0 (lhsT = per-head weight slice)
        q1T = sb.tile([D, H, IS], BF16, name="q1T", tag="q1T")
        k1T = sb.tile([D, H, J], BF16, name="k1T", tag="k1T")
        for h in range(H):
            hs = slice(h * D, (h + 1) * D)
            qp = pmm.tile([D, IS], F32, name="qp1", tag="mm")
            nc.tensor.matmul(qp, c_wq1t[:, hs], x1nT, start=True, stop=True)
            nc.scalar.activation(q1T[:, h, :], qp, ACTF.Copy, scale=ISCALE)
            for blk in range(2):
                kp = pmm.tile([D, 512], F32, name="kp1", tag="mm")
                nc.tensor.matmul(kp, c_wk1t[:, hs],
                                 x2nT[:, blk * 512:(blk + 1) * 512],
                                 start=True, stop=True)
                if h % 2 == 0:
                    nc.scalar.copy(k1T[:, h, blk * 512:(blk + 1) * 512], kp)
                else:
                    nc.vector.tensor_copy(k1T[:, h, blk * 512:(blk + 1) * 512], kp)

        # v1 [j, hd] (bf16) per j-tile
        v1 = sb.tile([128, 8, HD], BF16, name="v1", tag="v1")
        for jt in range(8):
            vp = pmm.tile([128, HD], F32, name="vp1", tag="mm")
            nc.tensor.matmul(vp, x2nT[:, jt * 128:(jt + 1) * 128], c_wv1t,
                             start=True, stop=True)
            nc.vector.tensor_copy(v1[:, jt, :], vp)

        # gating g1 = sigmoid(x1n @ wg1.T + bg1)   [i, hd]
        gp = pmm.tile([IS, HD], F32, name="gp1", tag="mm")
        nc.tensor.matmul(gp, x1nT, c_wg1t, start=True, stop=True)
        g1 = sb.tile([IS, HD], F32, name="g1", tag="g1")
        nc.vector.tensor_add(g1, gp, c_bg1b)
        nc.scalar.activation(g1, g1, ACTF.Sigmoid)

        q2T = sb.tile([D, H, J], BF16, name="q2T", tag="q2T")
        for h in range(H):
            hs = slice(h * D, (h + 1) * D)
            for blk in range(2):
                qp2 = pmm.tile([D, 512], F32, name="qp2", tag="mm")
                nc.tensor.matmul(qp2, c_wq2t[:, hs],
                                 x2nT[:, blk * 512:(blk + 1) * 512],
                                 start=True, stop=True)
                if h % 2 == 0:
                    nc.scalar.activation(q2T[:, h, blk * 512:(blk + 1) * 512],
                                         qp2, ACTF.Copy, scale=ISCALE)
                else:
                    nc.vector.tensor_scalar_mul(
                        q2T[:, h, blk * 512:(blk + 1) * 512], qp2, ISCALE)

        # reload per head as [i_part, j] (row 32k+h of group g -> i=4g+k)
        # and add mask bias -> combined bias per head
        _scr_r = tri_scr.rearrange("g (k r) j -> (g k) r j", k=IB)
        tribs = []
        for h in range(H):
            th = sb.tile([IS, J], BF16, name=f"trib{h}", tag=f"trib{h}")
            nc.sync.dma_start(th, _scr_r[:, h, :])
            nc.vector.tensor_add(th, th, mb)
            tribs.append(th)

        # ---- mha_1 ----
        l1 = sb.tile([IS, H], F32, name="l1", tag="l1")
        r1 = sb.tile([IS, H], F32, name="r1", tag="r1")
        o1n = sb.tile([IS, HD], F32, name="o1n", tag="o1n")
        for h in range(H):
            p1 = wk.tile([IS, J], BF16, name="p1", tag="p1")
            l1p = wk.tile([IS, 2], F32, name="l1p", tag="l1p")
            for blk in range(2):
                bs = slice(blk * 512, (blk + 1) * 512)
                sp = ptri.tile([IS, 512], F32, name="sp1", tag="tri")
                nc.tensor.matmul(sp, c_idbf, tribs[h][:, bs],
                                 start=True, stop=False)
                nc.tensor.matmul(sp, q1T[:, h, :], k1T[:, h, bs],
                                 start=False, stop=True)
                nc.scalar.activation(p1[:, bs], sp, ACTF.Exp,
                                     accum_out=l1p[:, blk:blk + 1])
            nc.vector.tensor_reduce(l1[:, h:h + 1], l1p, axis=AX.X, op=ALU.add)
            nc.vector.reciprocal(r1[:, h:h + 1], l1[:, h:h + 1])
            p1T = wk.tile([128, 8, IS], BF16, name="p1T", tag="p1T")
            for jt in range(8):
                tpb = ptp.tile([128, 128], BF16, name="tp_p1", tag="tp")
                nc.tensor.transpose(tpb, p1[:, jt * 128:(jt + 1) * 128], c_idbf)
                if jt % 2 == 0:
                    nc.vector.tensor_copy(p1T[:, jt, :], tpb)
                else:
                    nc.scalar.copy(p1T[:, jt, :], tpb)
            op = pacc.tile([IS, D], F32, name="op1", tag="acc")
            for jt in range(8):
                nc.tensor.matmul(op, p1T[:, jt, :], v1[:, jt, h * D:(h + 1) * D],
                                 start=(jt == 0), stop=(jt == 7))
            nc.scalar.activation(o1n[:, h * D:(h + 1) * D], op, ACTF.Copy,
                                 scale=r1[:, h:h + 1])

        og = sb.tile([IS, HD], F32, name="og", tag="og")
        nc.vector.tensor_mul(og, o1n, g1)
        ogT = sb.tile([128, 2, IS], F32, name="ogT", tag="ogT")
        for t in range(2):
            tp2 = ptp.tile([128, 128], F32, name="tp_og", tag="tp")
            nc.tensor.transpose(tp2, og[:, t * 128:(t + 1) * 128], c_id32)
            nc.vector.tensor_copy(ogT[:, t, :], tp2)

        xop = pacc.tile([C, IS], F32, name="xop", tag="acc")
        for t in range(2):
            nc.tensor.matmul(xop, c_wo1t[:, t, :], ogT[:, t, :],
                             start=(t == 0), stop=(t == 1))
        x1uT = sb.tile([C, IS], F32, name="x1uT", tag="x1uT")
        nc.scalar.activation(x1uT, xop, ACTF.Identity, bias=c_bo1c)
        nc.vector.tensor_add(x1uT, x1uT, x1nT)

        # x1u shard out (untransposed)
        tpo = ptp.tile([128, 128], F32, name="tp_x1u", tag="tp")
        nc.tensor.transpose(tpo, x1uT, c_id32)
        x1u_sb = sb.tile([IS, C], F32, name="x1u_sb", tag="x1u_sb")
        nc.vector.tensor_copy(x1u_sb, tpo)
        nc.sync.dma_start(x1u_o, x1u_sb)

        # ---- mha_2 partials over local keys ----
        k2T = sb.tile([D, H, IS], BF16, name="k2T", tag="k2T")
        for h in range(H):
            hs = slice(h * D, (h + 1) * D)
            kp2 = pmm.tile([D, IS], F32, name="kp2", tag="mm")
            nc.tensor.matmul(kp2, c_wk2t[:, hs], x1uT, start=True, stop=True)
            nc.scalar.copy(k2T[:, h, :], kp2)

        v2p = pmm.tile([IS, HD], F32, name="v2p", tag="mm")
        nc.tensor.matmul(v2p, x1uT, c_wv2t, start=True, stop=True)
        v2a = sb.tile([IS, H, D + 1], BF16, name="v2a", tag="v2a")
        nc.vector.memset(v2a, 1.0)
        for h in range(H):
            nc.vector.tensor_copy(v2a[:, h, :D], v2p[:, h * D:(h + 1) * D])

        for h in range(H):
            p2 = wk.tile([IS, J], BF16, name="p2", tag="p1")
            for blk in range(2):
                bs = slice(blk * 512, (blk + 1) * 512)
                sp2 = ptri.tile([IS, 512], F32, name="sp2", tag="tri")
                nc.tensor.matmul(sp2, c_idbf, tribs[h][:, bs],
                                 start=True, stop=False)
                nc.tensor.matmul(sp2, k2T[:, h, :], q2T[:, h, bs],
                                 start=False, stop=True)
                nc.scalar.activation(p2[:, bs], sp2, ACTF.Exp)
            o2h = wk.tile([D + 1, J], F32, name="o2h", tag="o2h")
            for blk in range(2):
                o2ps = pmm.tile([D + 1, 512], F32, name="o2ps", tag="mm")
                nc.tensor.matmul(o2ps, v2a[:, h, :],
                                 p2[:, blk * 512:(blk + 1) * 512],
                                 start=True, stop=True)
                if blk == 0:
                    nc.vector.tensor_copy(o2h[:, :512], o2ps)
                else:
                    nc.scalar.copy(o2h[:, 512:], o2ps)
            nc.sync.dma_start(o2p_o[h], o2h)

        for p in reversed((cst, sb, wk, xpp, stp, drp, ptri, ptp, pmm, pacc)):
            p.release()

    nc.compile()
    return nc


_CACHE = {}


def _get_program():
    if "nc" not in _CACHE:
        _CACHE["nc"] = build_program()
    return _CACHE["nc"]


def _np_ln(x):
    mu = x.mean(-1, keepdims=True)
    var = np.square(x - mu).mean(-1, keepdims=True)
    return (x - mu) / np.sqrt(var + EPS)


def make_in_maps(x1, x2, x_pair, mask, ln_w, ln_b, wb,
                 wq1, wk1, wv1, wg1, bg1, wo1, bo1,
                 wq2, wk2, wv2, wg2, bg2, wo2, bo2):
    f = np.float32
    shared = {
        "x2d": np.ascontiguousarray(x2[0], dtype=f),
        "lnw": np.tile(np.asarray(ln_w, f), (128, 1)),
        "lnb": np.tile(np.asarray(ln_b, f), (128, 1)),
        "wq1t": np.ascontiguousarray(np.asarray(wq1, f).T),
        "wk1t": np.ascontiguousarray(np.asarray(wk1, f).T).astype(BF),
        "wv1t": np.ascontiguousarray(np.asarray(wv1, f).T).astype(BF),
        "wg1t": np.ascontiguousarray(np.asarray(wg1, f).T),
        "wo1t": np.ascontiguousarray(np.asarray(wo1, f).T),
        "bg1b": np.tile(np.asarray(bg1, f), (128, 1)),
        "bo1c": np.asarray(bo1, f)[:, None].copy(),
        "wq2t": np.ascontiguousarray(np.asarray(wq2, f).T).astype(BF),
        "wk2t": np.ascontiguousarray(np.asarray(wk2, f).T),
        "wv2t": np.ascontiguousarray(np.asarray(wv2, f).T),
        "wbt": np.tile(np.ascontiguousarray(np.asarray(wb, f).T), (1, 4)).astype(BF),
        "id32": np.eye(128, dtype=f),
        "idbf": np.eye(128, dtype=f).astype(BF),
    }
    in_maps = []
    x1np = np.asarray(x1, f)
    xpnp = np.asarray(x_pair, f)
    msknp = np.asarray(mask, f)
    for m in range(NCORES):
        sl = slice(m * IS, (m + 1) * IS)
        im = dict(shared)
        im["x1s"] = np.ascontiguousarray(x1np[0, sl])
        im["msk"] = np.ascontiguousarray(msknp[0, sl])
        im["xpt"] = np.ascontiguousarray(
            xpnp[0, sl].transpose(0, 2, 1)).astype(BF)
        in_maps.append(im)
    return in_maps


def combine(results, x2, wg2, bg2, wo2, bo2):
    f = np.float32
    x1u = np.concatenate([results[m]["x1u_o"] for m in range(NCORES)],
                         axis=0)[None]
    o2p = np.sum([results[m]["o2p_o"].astype(np.float64)
                  for m in range(NCORES)], axis=0)
    o2 = o2p[:, :D, :]                    # [H, D, J]
    l2 = o2p[:, D, :]                     # [H, J]
    on = (o2 / l2[:, None, :]).astype(f)
    o_fl = on.transpose(2, 0, 1).reshape(J, HD)       # [j, hd]
    x2n = _np_ln(np.asarray(x2[0], f))
    g2 = 1.0 / (1.0 + np.exp(-(x2n @ np.asarray(wg2, f).T
                               + np.asarray(bg2, f))))
    x2u = x2n + (o_fl * g2) @ np.asarray(wo2, f).T + np.asarray(bo2, f)
    return x1u.astype(f), x2u[None].astype(f)


def kernel(**inputs):
    nc = _get_program()
    in_maps = make_in_maps(**inputs)
    res = run_bass_kernel_spmd(nc, in_maps, core_ids=list(range(NCORES)))
    return combine(res.results, inputs["x2"], inputs["wg2"], inputs["bg2"],
                   inputs["wo2"], inputs["bo2"])


if __name__ == "__main__":
    import reference
    inputs = {k: np.asarray(v) for k, v in reference.setup_inputs().items()}
    e1, e2 = reference.reference(**inputs)
    a1, a2 = kernel(**inputs)
    for name, e, a in (("x1u", e1, a1), ("x2u", e2, a2)):
        e = np.asarray(e)
        err = np.abs(a - e).max() / (np.abs(e).max() + 1e-12)
        print(f"{name}: rel_err={err:.3e}")



# revision 14
# speedup vs baseline: 1.2996x; 1.2996x over previous
"""BiDirectionalTriangleAttention on 8 TRN2 NeuronCores (Bass/Tile SPMD).

Sharding: I (row) axis of x1/x_pair/mask split across 8 cores (128 rows each).
Per core:
  - triangle bias tri[h, i_loc, j] from a host-pre-transposed fp8 x_pair shard
    ([i, c, j] layout) so the C contraction lands on SBUF partitions.  4-row
    col-packed matmul quads -> PSUM -> bf16 SBUF staging -> SBUF->SBUF DMA
    relayout into [i_part, h, j].
  - LayerNorm of x1/x2 done on host (host needs x2n for the x2u finish anyway);
    device receives x1nT/x2nT directly.
  - mha_1 fully local (queries = local rows, keys = full x2n).  Scores via
    identity-seeded PSUM (bias) + 4-head row-packed QK matmuls; softmax
    denominators via a ones-column appended to V.
  - mha_2 flash-style partials over the local key rows (keys/values = locally
    updated x1u), 4-head col-packed AV + ones-lhsT exp-sum matmuls.  Host
    merges the 8 partials and applies gating + output projection for x2u.
"""

import numpy as np
import ml_dtypes

import concourse.bass as bass
import concourse.bacc as bacc
import concourse.mybir as mybir
import concourse.tile as tile
from concourse.bass_utils import run_bass_kernel_spmd

F32 = mybir.dt.float32
BF16 = mybir.dt.bfloat16
F8 = mybir.dt.float8e4
BF = ml_dtypes.bfloat16
F8NP = ml_dtypes.float8_e4m3
AX = mybir.AxisListType
ALU = mybir.AluOpType
ACTF = mybir.ActivationFunctionType

B, I, J, C, H, D = 1, 1024, 1024, 128, 8, 32
HD = H * D          # 256
NCORES = 8
IS = I // NCORES    # 128 rows per core
INF = 1e9
EPS = 1e-5
ISCALE = float(1.0 / np.sqrt(np.float32(D)))

GR = 16             # x_pair rows per DMA (2 MiB fp8)
NG = IS // 4        # 32 quad groups of 4 rows

# trib partition p = 32k+g holds local i-row 4g+k
PERM = np.array([4 * (p % 32) + p // 32 for p in range(IS)])   # p -> i
IPERM = np.argsort(PERM)                                       # i -> p

# wcat column layout (bf16)
_WOFF = {}
_off = 0
for _nm, _w in (("wq1", HD), ("wk1", HD), ("wv1", HD), ("wg1", HD),
                ("wo1", HD), ("wq2", HD), ("wk2", HD), ("wv2", HD),
                ("wbt", 4 * H), ("idbf", 128), ("ones", 1)):
    _WOFF[_nm] = (_off, _off + _w)
    _off += _w
NW = _off           # total wcat cols


def build_program():
    nc = bacc.Bacc("TRN2", target_bir_lowering=False, debug=False,
                   num_devices=NCORES)

    def din(name, shape, dt=F32):
        return nc.dram_tensor(name, shape, dt, kind="ExternalInput").ap()

    def dout(name, shape, dt=F32):
        return nc.dram_tensor(name, shape, dt, kind="ExternalOutput").ap()

    xpt = din("xpt", [IS, C, J], F8)       # x_pair shard, [i, c, j] fp8
    wcat = din("wcat", [128, NW], BF16)    # all bf16 weights, concatenated
    fcat = din("fcat", [128, 129 + HD])    # id32 | bo1 | bg1b (f32)
    x2nTd = din("x2nTd", [C, J], BF16)     # LN(x2)^T (host)
    x1nTd = din("x1nTd", [C, IS])          # LN(x1)^T shard (host, f32)
    mbb = din("mbb", [IS, J], BF16)        # INF*(mask-1) shard

    x1u_o = dout("x1u_o", [IS, C])
    o2_o = dout("o2_o", [2, 2, 128, 512], BF16)   # [hg, blk, (m d), j]
    l2_o = dout("l2_o", [2, 2, 4, 512], BF16)     # [hg, blk, m, j]

    with tile.TileContext(nc) as tc:
        cst = tc.alloc_tile_pool(name="cst", bufs=1)
        sb = tc.alloc_tile_pool(name="sb", bufs=1)
        xtp = tc.alloc_tile_pool(name="xtp", bufs=3)
        wk = tc.alloc_tile_pool(name="wk", bufs=2)
        pquad = tc.alloc_tile_pool(name="pquad", bufs=2, space="PSUM")
        pmid = tc.alloc_tile_pool(name="pmid", bufs=2, space="PSUM")
        ptp = tc.alloc_tile_pool(name="ptp", bufs=2, space="PSUM")

        # ---- input DMAs ----
        xts = []
        for gg in range(8):
            xt = xtp.tile([C, GR, J], F8, name=f"xt{gg}", tag="xt")
            nc.sync.dma_start(xt, xpt[gg * GR:(gg + 1) * GR]
                              .rearrange("i c j -> c i j"))
            xts.append(xt)

        wsb = cst.tile([128, NW], BF16, name="wsb", tag="wsb")
        nc.sync.dma_start(wsb, wcat)
        fsb = cst.tile([128, 129 + HD], F32, name="fsb", tag="fsb")
        nc.sync.dma_start(fsb, fcat)
        x2nT = cst.tile([C, J], BF16, name="x2nT", tag="x2nT")
        nc.sync.dma_start(x2nT, x2nTd)
        x1nT = cst.tile([C, IS], F32, name="x1nT", tag="x1nT")
        nc.sync.dma_start(x1nT, x1nTd)
        mbs = cst.tile([IS, J], BF16, name="mbs", tag="mbs")
        nc.sync.dma_start(mbs, mbb)

        def w(nm):
            lo, hi = _WOFF[nm]
            return wsb[:, lo:hi]

        c_idbf = w("idbf")
        c_id32 = fsb[:, 0:128]
        c_bo1 = fsb[:, 128:129]
        c_bg1b = fsb[:, 129:129 + HD]

        # const ap for activation bias literals
        for cval in (0.0,):
            cap = cst.tile([128, 1], F32, name=f"cap{cval}", tag=f"cap{cval}")
            nc.vector.memset(cap, cval)
            nc.const_aps.aps[(F32, cval)] = cap

        x1nTb = sb.tile([C, IS], BF16, name="x1nTb", tag="x1nTb")
        nc.vector.tensor_copy(x1nTb, x1nT)

        # ---- projections (fill PE while x_pair streams) ----
        # q1T/k1T/q2T/k2T packed: partition 32*(h%4)+d, second index hg=h//4
        q1T = sb.tile([128, 2, IS], BF16, name="q1T", tag="q1T")
        k1T = sb.tile([128, 2, J], BF16, name="k1T", tag="k1T")
        q2T = sb.tile([128, 2, J], BF16, name="q2T", tag="q2T")
        for hg in range(2):
            hs = slice(hg * 128, (hg + 1) * 128)
            qp = pmid.tile([128, IS], F32, name="qp1", tag="mid")
            nc.tensor.matmul(qp, w("wq1")[:, hs], x1nTb, start=True, stop=True)
            nc.scalar.copy(q1T[:, hg, :], qp)
            for blk in range(2):
                bs = slice(blk * 512, (blk + 1) * 512)
                kp = pmid.tile([128, 512], F32, name="kp1", tag="mid")
                nc.tensor.matmul(kp, w("wk1")[:, hs], x2nT[:, bs],
                                 start=True, stop=True)
                if blk == 0:
                    nc.vector.tensor_copy(k1T[:, hg, bs], kp)
                else:
                    nc.scalar.copy(k1T[:, hg, bs], kp)
                qp2 = pmid.tile([128, 512], F32, name="qp2", tag="mid")
                nc.tensor.matmul(qp2, w("wq2")[:, hs], x2nT[:, bs],
                                 start=True, stop=True)
                if blk == 0:
                    nc.scalar.copy(q2T[:, hg, bs], qp2)
                else:
                    nc.vector.tensor_copy(q2T[:, hg, bs], qp2)

        # v1 with ones column appended: [j, jt, h, D+1]
        v1a = sb.tile([128, 8, H, D + 1], BF16, name="v1a", tag="v1a")
        nc.gpsimd.memset(v1a, 1.0)
        for jt in range(8):
            vp = pmid.tile([128, HD], F32, name="vp1", tag="mid")
            nc.tensor.matmul(vp, x2nT[:, jt * 128:(jt + 1) * 128], w("wv1"),
                             start=True, stop=True)
            dst = v1a[:, jt, :, 0:D]
            if jt % 2 == 0:
                nc.vector.tensor_copy(dst, vp.rearrange("p (h d) -> p h d", h=H))
            else:
                nc.scalar.copy(dst, vp.rearrange("p (h d) -> p h d", h=H))

        # gating g1 = sigmoid(x1n @ wg1.T + bg1)
        gp = pmid.tile([IS, HD], F32, name="gp1", tag="mid")
        nc.tensor.matmul(gp, x1nTb, w("wg1"), start=True, stop=True)
        g1 = sb.tile([IS, HD], F32, name="g1", tag="g1")
        nc.vector.tensor_add(g1, gp, c_bg1b)
        nc.scalar.activation(g1, g1, ACTF.Sigmoid)

        # ---- triangle bias: fp8 stream -> quads -> bf16 staging ----
        stg = sb.tile([128, NG, J], BF16, name="stg", tag="stg")
        for gg in range(8):
            xt = xts[gg]
            for q in range(4):
                g = gg * 4 + q
                quad = pquad.tile([128, J], F32, name="quad", tag="quad")
                for blk in range(2):
                    bs = slice(blk * 512, (blk + 1) * 512)
                    for k in range(4):
                        nc.tensor.matmul(
                            quad[32 * k:32 * (k + 1), bs], w("wbt"),
                            xt[:, q * 4 + k, bs], start=True, stop=True,
                            tile_position=(0, 32 * k))
                if g % 2 == 0:
                    nc.vector.tensor_copy(stg[:, g, :], quad)
                else:
                    nc.scalar.copy(stg[:, g, :], quad)

        # relayout: stg[32k+h, g, j] -> trib[32k+g, h, j]  (SBUF->SBUF DMA).
        # trib rows are PERMUTED: partition p = 32k+g holds i-row 4g+k.  The
        # host permutes x1nT/mbb columns/rows to match and inverse-permutes
        # the x1u output; o2/l2 sum over i so they are unaffected.
        trib = sb.tile([IS, H, J], BF16, name="trib", tag="trib")
        for k in range(4):
            for h in range(H):
                nc.sync.dma_start(trib[32 * k:32 * k + 32, h, :],
                                  stg[32 * k + h:32 * k + h + 1])
        for h in range(H):
            nc.gpsimd.tensor_add(trib[:, h, :], trib[:, h, :], mbs)

        # ---- mha_1 ----
        o1n = sb.tile([IS, HD], F32, name="o1n", tag="o1n")
        for hg in range(2):
            p1s = [wk.tile([IS, J], BF16, name=f"p1_{hg}_{m}", tag=f"p1{m}",
                           bufs=1) for m in range(4)]
            for blk in range(2):
                bs = slice(blk * 512, (blk + 1) * 512)
                qa = pquad.tile([128, J], F32, name="qa", tag="quad")
                banks = [qa[:, 0:512], qa[:, 512:1024]]
                qb = pquad.tile([128, J], F32, name="qb", tag="quad")
                banks += [qb[:, 0:512], qb[:, 512:1024]]
                for m in range(4):
                    nc.tensor.matmul(banks[m], c_idbf,
                                     trib[:, hg * 4 + m, bs],
                                     start=True, stop=False)
                for m in range(4):
                    nc.tensor.matmul(banks[m], q1T[32 * m:32 * (m + 1), hg, :],
                                     k1T[32 * m:32 * (m + 1), hg, bs],
                                     start=False, stop=True,
                                     tile_position=(32 * m, 0))
                for m in range(4):
                    nc.scalar.activation(p1s[m][:, bs], banks[m], ACTF.Exp)
            for m in range(4):
                h = hg * 4 + m
                p1T = wk.tile([128, 8, IS], BF16, name="p1T", tag="p1T")
                for jt in range(8):
                    tp = ptp.tile([128, 128], BF16, name="tp1", tag="tp")
                    nc.tensor.transpose(tp, p1s[m][:, jt * 128:(jt + 1) * 128],
                                        c_idbf)
                    if jt % 2 == 0:
                        nc.vector.tensor_copy(p1T[:, jt, :], tp)
                    else:
                        nc.scalar.copy(p1T[:, jt, :], tp)
                acc = pmid.tile([IS, D + 1], F32, name="acc1", tag="mid")
                for jt in range(8):
                    nc.tensor.matmul(acc, p1T[:, jt, :], v1a[:, jt, h, :],
                                     start=(jt == 0), stop=(jt == 7))
                r1 = wk.tile([IS, 1], F32, name="r1", tag="r1")
                nc.vector.reciprocal(r1, acc[:, D:D + 1])
                nc.scalar.activation(o1n[:, h * D:(h + 1) * D], acc[:, 0:D],
                                     ACTF.Copy, scale=r1)

        # ---- x1u = x1n + wo1 @ (o1 * g1) + bo1 ----
        og = sb.tile([IS, HD], F32, name="og", tag="og")
        nc.vector.tensor_mul(og, o1n, g1)
        ogT = sb.tile([128, 2, IS], BF16, name="ogT", tag="ogT")
        for t in range(2):
            tp2 = ptp.tile([128, 128], F32, name="tp_og", tag="tp")
            nc.tensor.transpose(tp2, og[:, t * 128:(t + 1) * 128], c_id32)
            nc.vector.tensor_copy(ogT[:, t, :], tp2)
        xop = pmid.tile([C, IS], F32, name="xop", tag="mid")
        for t in range(2):
            nc.tensor.matmul(xop, w("wo1")[:, t * 128:(t + 1) * 128],
                             ogT[:, t, :], start=(t == 0), stop=(t == 1))
        x1uT = sb.tile([C, IS], F32, name="x1uT", tag="x1uT")
        nc.scalar.activation(x1uT, xop, ACTF.Identity, bias=c_bo1)
        nc.vector.tensor_add(x1uT, x1uT, x1nT)
        x1uTb = sb.tile([C, IS], BF16, name="x1uTb", tag="x1uTb")
        nc.vector.tensor_copy(x1uTb, x1uT)

        tpo = ptp.tile([128, 128], F32, name="tp_x1u", tag="tp")
        nc.tensor.transpose(tpo, x1uT, c_id32)
        x1u_sb = sb.tile([IS, C], F32, name="x1u_sb", tag="x1u_sb")
        nc.scalar.copy(x1u_sb, tpo)
        nc.sync.dma_start(x1u_o, x1u_sb)

        # ---- mha_2 projections from x1u ----
        k2T = sb.tile([128, 2, IS], BF16, name="k2T", tag="k2T")
        for hg in range(2):
            kp2 = pmid.tile([128, IS], F32, name="kp2", tag="mid")
            nc.tensor.matmul(kp2, w("wk2")[:, hg * 128:(hg + 1) * 128], x1uTb,
                             start=True, stop=True)
            nc.scalar.copy(k2T[:, hg, :], kp2)
        vp2 = pmid.tile([IS, HD], F32, name="vp2", tag="mid")
        nc.tensor.matmul(vp2, x1uTb, w("wv2"), start=True, stop=True)
        v2sb = sb.tile([IS, HD], BF16, name="v2sb", tag="v2sb")
        nc.vector.tensor_copy(v2sb, vp2)

        # ---- mha_2 partials over local keys ----
        for hg in range(2):
            for blk in range(2):
                bs = slice(blk * 512, (blk + 1) * 512)
                qa = pquad.tile([128, J], F32, name="qa2", tag="quad")
                banks = [qa[:, 0:512], qa[:, 512:1024]]
                qb = pquad.tile([128, J], F32, name="qb2", tag="quad")
                banks += [qb[:, 0:512], qb[:, 512:1024]]
                for m in range(4):
                    nc.tensor.matmul(banks[m], c_idbf,
                                     trib[:, hg * 4 + m, bs],
                                     start=True, stop=False)
                for m in range(4):
                    nc.tensor.matmul(banks[m], k2T[32 * m:32 * (m + 1), hg, :],
                                     q2T[32 * m:32 * (m + 1), hg, bs],
                                     start=False, stop=True,
                                     tile_position=(32 * m, 0))
                p2 = wk.tile([IS, 4, 512], BF16, name="p2", tag="p2")
                for m in range(4):
                    nc.scalar.activation(p2[:, m, :], banks[m], ACTF.Exp)
                av = pmid.tile([128, 512], F32, name="av", tag="mid")
                for m in range(4):
                    h = hg * 4 + m
                    nc.tensor.matmul(av[32 * m:32 * (m + 1), :],
                                     v2sb[:, h * D:(h + 1) * D], p2[:, m, :],
                                     start=True, stop=True,
                                     tile_position=(0, 32 * m))
                l2b = pmid.tile([128, 512], F32, name="l2b", tag="mid")
                for m in range(4):
                    nc.tensor.matmul(l2b[32 * m:32 * m + 1, :], w("ones"),
                                     p2[:, m, :], start=True, stop=True,
                                     tile_position=(0, 32 * m))
                o2sb = wk.tile([128, 512], BF16, name="o2sb", tag="o2sb")
                nc.vector.tensor_copy(o2sb, av)
                nc.sync.dma_start(o2_o[hg, blk], o2sb)
                l2sb = wk.tile([128, 512], BF16, name="l2sb", tag="l2sb")
                nc.scalar.copy(l2sb, l2b)
                l2v = l2sb.rearrange("(m r) j -> m r j", m=4)
                nc.sync.dma_start(l2_o[hg, blk], l2v[:, 0, :])

        for p in reversed((cst, sb, xtp, wk, pquad, pmid, ptp)):
            p.release()

    nc.compile()
    return nc


_CACHE = {}


def _get_program():
    if "nc" not in _CACHE:
        _CACHE["nc"] = build_program()
    return _CACHE["nc"]


def _np_ln(x):
    mu = x.mean(-1, keepdims=True)
    var = np.square(x - mu).mean(-1, keepdims=True)
    return (x - mu) / np.sqrt(var + EPS)


def make_in_maps(x1, x2, x_pair, mask, ln_w, ln_b, wb,
                 wq1, wk1, wv1, wg1, bg1, wo1, bo1,
                 wq2, wk2, wv2, wg2, bg2, wo2, bo2):
    f = np.float32

    def t(a):
        return np.ascontiguousarray(np.asarray(a, f).T)

    lw = np.asarray(ln_w, f)
    lb = np.asarray(ln_b, f)
    x1n = (_np_ln(np.asarray(x1[0], f)) * lw + lb).astype(f)
    x2n = (_np_ln(np.asarray(x2[0], f)) * lw + lb).astype(f)

    wo1t = t(wo1)  # [HD, C]
    blocks = [
        (t(wq1) * ISCALE), t(wk1), t(wv1), t(wg1),
        wo1t.reshape(2, 128, C).transpose(1, 0, 2).reshape(128, 2 * C),
        (t(wq2) * ISCALE), t(wk2), t(wv2),
        np.tile(t(wb), (1, 4)),
        np.eye(128, dtype=f),
        np.ones((128, 1), dtype=f),
    ]
    wcat = np.concatenate(blocks, axis=1).astype(BF)
    assert wcat.shape[1] == NW, (wcat.shape, NW)
    fcat = np.concatenate(
        [np.eye(128, dtype=f), np.asarray(bo1, f)[:, None],
         np.tile(np.asarray(bg1, f), (128, 1))], axis=1)

    shared = {
        "wcat": wcat,
        "fcat": np.ascontiguousarray(fcat),
        "x2nTd": np.ascontiguousarray(x2n.T).astype(BF),
    }
    in_maps = []
    xpnp = np.asarray(x_pair, f)
    msknp = np.asarray(mask, f)
    for m in range(NCORES):
        sl = slice(m * IS, (m + 1) * IS)
        im = dict(shared)
        im["x1nTd"] = np.ascontiguousarray(x1n[sl][PERM].T)
        im["mbb"] = np.ascontiguousarray(
            (INF * (msknp[0, sl] - 1.0))[PERM]).astype(BF)
        im["xpt"] = np.ascontiguousarray(
            xpnp[0, sl].transpose(0, 2, 1)).astype(F8NP)
        in_maps.append(im)
    return in_maps


def combine(results, x2, wg2, bg2, wo2, bo2):
    f = np.float32
    x1u = np.concatenate([results[m]["x1u_o"][IPERM] for m in range(NCORES)],
                         axis=0)[None]
    o2 = np.zeros((H, D, J), dtype=np.float64)
    l2 = np.zeros((H, J), dtype=np.float64)
    for m in range(NCORES):
        o2p = results[m]["o2_o"].astype(np.float64)  # [hg, blk, 128, 512]
        l2p = results[m]["l2_o"].astype(np.float64)  # [hg, blk, 4, 512]
        for hg in range(2):
            for blk in range(2):
                js = slice(blk * 512, (blk + 1) * 512)
                o2[hg * 4:(hg + 1) * 4, :, js] += \
                    o2p[hg, blk].reshape(4, D, 512)
                l2[hg * 4:(hg + 1) * 4, js] += l2p[hg, blk]
    on = (o2 / l2[:, None, :]).astype(f)
    o_fl = on.transpose(2, 0, 1).reshape(J, HD)       # [j, hd]
    x2n = _np_ln(np.asarray(x2[0], f))
    g2 = 1.0 / (1.0 + np.exp(-(x2n @ np.asarray(wg2, f).T
                               + np.asarray(bg2, f))))
    x2u = x2n + (o_fl * g2) @ np.asarray(wo2, f).T + np.asarray(bo2, f)
    return x1u.astype(f), x2u[None].astype(f)


def kernel(**inputs):
    nc = _get_program()
    in_maps = make_in_maps(**inputs)
    res = run_bass_kernel_spmd(nc, in_maps, core_ids=list(range(NCORES)))
    return combine(res.results, inputs["x2"], inputs["wg2"], inputs["bg2"],
                   inputs["wo2"], inputs["bo2"])


if __name__ == "__main__":
    import reference
    inputs = {k: np.asarray(v) for k, v in reference.setup_inputs().items()}
    e1, e2 = reference.reference(**inputs)
    a1, a2 = kernel(**inputs)
    for name, e, a in (("x1u", e1, a1), ("x2u", e2, a2)):
        e = np.asarray(e)
        err = np.abs(a - e).max() / (np.abs(e).max() + 1e-12)
        print(f"{name}: rel_err={err:.3e}")


# revision 18
# speedup vs baseline: 1.4045x; 1.0807x over previous
"""BiDirectionalTriangleAttention on 8 TRN2 NeuronCores (Bass/Tile SPMD).

Sharding: I (row) axis of x1/x_pair/mask split across 8 cores (128 rows each).
Per core:
  - triangle bias tri[h, i_loc, j] from a host-pre-transposed fp8 x_pair shard
    ([i, c, j] layout) so the C contraction lands on SBUF partitions.  4-row
    col-packed matmul quads -> PSUM -> bf16 SBUF staging -> SBUF->SBUF DMA
    relayout into [i_part, h, j].
  - LayerNorm of x1/x2 done on host (host needs x2n for the x2u finish anyway);
    device receives x1nT/x2nT directly.
  - mha_1 fully local (queries = local rows, keys = full x2n).  Scores via
    identity-seeded PSUM (bias) + 4-head row-packed QK matmuls; softmax
    denominators via a ones-column appended to V.
  - mha_2 flash-style partials over the local key rows (keys/values = locally
    updated x1u), 4-head col-packed AV + ones-lhsT exp-sum matmuls.  Host
    merges the 8 partials and applies gating + output projection for x2u.
"""

import numpy as np
import ml_dtypes

import concourse.bass as bass
import concourse.bacc as bacc
import concourse.mybir as mybir
import concourse.tile as tile
from concourse.bass_utils import run_bass_kernel_spmd

F32 = mybir.dt.float32
BF16 = mybir.dt.bfloat16
F8 = mybir.dt.float8e4
BF = ml_dtypes.bfloat16
F8NP = ml_dtypes.float8_e4m3
AX = mybir.AxisListType
ALU = mybir.AluOpType
ACTF = mybir.ActivationFunctionType

B, I, J, C, H, D = 1, 1024, 1024, 128, 8, 32
HD = H * D          # 256
NCORES = 8
IS = I // NCORES    # 128 rows per core
INF = 1e9
EPS = 1e-5
ISCALE = float(1.0 / np.sqrt(np.float32(D)))

GR = 16             # x_pair rows per DMA (2 MiB fp8)
NG = IS // 4        # 32 quad groups of 4 rows

# trib partition p = 32k+g holds local i-row 4g+k
PERM = np.array([4 * (p % 32) + p // 32 for p in range(IS)])   # p -> i
IPERM = np.argsort(PERM)                                       # i -> p

# wcat column layout (bf16)
_WOFF = {}
_off = 0
for _nm, _w in (("wq1", HD), ("wk1", HD), ("wv1", HD), ("wg1", HD),
                ("wo1", HD), ("wq2", HD), ("wk2", HD), ("wv2", HD),
                ("wbt", 4 * H), ("idbf", 128), ("ones", 1)):
    _WOFF[_nm] = (_off, _off + _w)
    _off += _w
NW = _off           # total wcat cols


def build_program():
    nc = bacc.Bacc("TRN2", target_bir_lowering=False, debug=False,
                   num_devices=NCORES)

    def din(name, shape, dt=F32):
        return nc.dram_tensor(name, shape, dt, kind="ExternalInput").ap()

    def dout(name, shape, dt=F32):
        return nc.dram_tensor(name, shape, dt, kind="ExternalOutput").ap()

    xpt = din("xpt", [IS, C, J], F8)       # x_pair shard, [i, c, j] fp8
    wcat = din("wcat", [128, NW], BF16)    # all bf16 weights, concatenated
    fcat = din("fcat", [128, 129 + HD])    # id32 | bo1 | bg1b (f32)
    x2nTd = din("x2nTd", [C, J], BF16)     # LN(x2)^T (host)
    x1nTd = din("x1nTd", [C, IS])          # LN(x1)^T shard (host, f32)
    mbb = din("mbb", [IS, J], BF16)        # INF*(mask-1) shard

    x1u_o = dout("x1u_o", [IS, C])
    o2_o = dout("o2_o", [2, 2, 128, 512], BF16)   # [hg, blk, (m d), j]
    l2_o = dout("l2_o", [2, 2, 4, 512], BF16)     # [hg, blk, m, j]

    with tile.TileContext(nc) as tc:
        cst = tc.alloc_tile_pool(name="cst", bufs=1)
        sb = tc.alloc_tile_pool(name="sb", bufs=1)
        xtp = tc.alloc_tile_pool(name="xtp", bufs=3)
        wk = tc.alloc_tile_pool(name="wk", bufs=2)
        drp = tc.alloc_tile_pool(name="drp", bufs=1, space="DRAM")
        pquad = tc.alloc_tile_pool(name="pquad", bufs=2, space="PSUM")
        pmid = tc.alloc_tile_pool(name="pmid", bufs=2, space="PSUM")
        ptp = tc.alloc_tile_pool(name="ptp", bufs=2, space="PSUM")

        # ---- input DMAs ----
        # consts on the scalar HWDGE ring so they land in parallel with the
        # x_pair stream (sync ring) and the first matmuls start early
        wsb = cst.tile([128, NW], BF16, name="wsb", tag="wsb")
        nc.scalar.dma_start(wsb, wcat)
        fsb = cst.tile([128, 129 + HD], F32, name="fsb", tag="fsb")
        nc.scalar.dma_start(fsb, fcat)
        x2nT = cst.tile([C, J], BF16, name="x2nT", tag="x2nT")
        nc.scalar.dma_start(x2nT, x2nTd)
        x1nT = cst.tile([C, IS], F32, name="x1nT", tag="x1nT")
        nc.scalar.dma_start(x1nT, x1nTd)
        mbs = cst.tile([IS, J], BF16, name="mbs", tag="mbs")
        nc.scalar.dma_start(mbs, mbb)

        xts = []
        for gg in range(8):
            xt = xtp.tile([C, GR, J], F8, name=f"xt{gg}", tag="xt")
            nc.sync.dma_start(xt, xpt[gg * GR:(gg + 1) * GR]
                              .rearrange("i c j -> c i j"))
            xts.append(xt)

        def w(nm):
            lo, hi = _WOFF[nm]
            return wsb[:, lo:hi]

        c_idbf = w("idbf")
        c_id32 = fsb[:, 0:128]
        c_bo1 = fsb[:, 128:129]
        c_bg1b = fsb[:, 129:129 + HD]

        # const ap for activation bias literals
        for cval in (0.0,):
            cap = cst.tile([128, 1], F32, name=f"cap{cval}", tag=f"cap{cval}")
            nc.vector.memset(cap, cval)
            nc.const_aps.aps[(F32, cval)] = cap

        x1nTb = sb.tile([C, IS], BF16, name="x1nTb", tag="x1nTb")
        nc.vector.tensor_copy(x1nTb, x1nT)

        # ---- projections (fill PE while x_pair streams) ----
        # q1T/k1T/q2T/k2T packed: partition 32*(h%4)+d, second index hg=h//4
        q1T = sb.tile([128, 2, IS], BF16, name="q1T", tag="q1T")
        k1T = sb.tile([128, 2, J], BF16, name="k1T", tag="k1T")
        q2T = sb.tile([128, 2, J], BF16, name="q2T", tag="q2T")
        for hg in range(2):
            hs = slice(hg * 128, (hg + 1) * 128)
            qp = pmid.tile([128, IS], F32, name="qp1", tag="mid")
            nc.tensor.matmul(qp, w("wq1")[:, hs], x1nTb, start=True, stop=True)
            nc.scalar.copy(q1T[:, hg, :], qp)
            for blk in range(2):
                bs = slice(blk * 512, (blk + 1) * 512)
                kp = pmid.tile([128, 512], F32, name="kp1", tag="mid")
                nc.tensor.matmul(kp, w("wk1")[:, hs], x2nT[:, bs],
                                 start=True, stop=True)
                if blk == 0:
                    nc.vector.tensor_copy(k1T[:, hg, bs], kp)
                else:
                    nc.scalar.copy(k1T[:, hg, bs], kp)
                qp2 = pmid.tile([128, 512], F32, name="qp2", tag="mid")
                nc.tensor.matmul(qp2, w("wq2")[:, hs], x2nT[:, bs],
                                 start=True, stop=True)
                if blk == 0:
                    nc.scalar.copy(q2T[:, hg, bs], qp2)
                else:
                    nc.vector.tensor_copy(q2T[:, hg, bs], qp2)

        # v1 with ones column appended: [j, jt, h, D+1]
        v1a = sb.tile([128, 8, H, D + 1], BF16, name="v1a", tag="v1a")
        nc.gpsimd.memset(v1a, 1.0)
        for jt in range(8):
            vp = pmid.tile([128, HD], F32, name="vp1", tag="mid")
            nc.tensor.matmul(vp, x2nT[:, jt * 128:(jt + 1) * 128], w("wv1"),
                             start=True, stop=True)
            dst = v1a[:, jt, :, 0:D]
            if jt % 2 == 0:
                nc.vector.tensor_copy(dst, vp.rearrange("p (h d) -> p h d", h=H))
            else:
                nc.scalar.copy(dst, vp.rearrange("p (h d) -> p h d", h=H))

        # gating g1 = sigmoid(x1n @ wg1.T + bg1)
        gp = pmid.tile([IS, HD], F32, name="gp1", tag="mid")
        nc.tensor.matmul(gp, x1nTb, w("wg1"), start=True, stop=True)
        g1 = sb.tile([IS, HD], F32, name="g1", tag="g1")
        nc.vector.tensor_add(g1, gp, c_bg1b)
        nc.scalar.activation(g1, g1, ACTF.Sigmoid)

        # ---- triangle bias: fp8 stream -> quads -> bf16 staging ----
        stg = sb.tile([128, NG, J], BF16, name="stg", tag="stg")
        for gg in range(8):
            xt = xts[gg]
            for q in range(4):
                g = gg * 4 + q
                quad = pquad.tile([128, J], F32, name="quad", tag="quad")
                for blk in range(2):
                    bs = slice(blk * 512, (blk + 1) * 512)
                    for k in range(4):
                        nc.tensor.matmul(
                            quad[32 * k:32 * (k + 1), bs], w("wbt"),
                            xt[:, q * 4 + k, bs], start=True, stop=True,
                            tile_position=(0, 32 * k))
                if g % 2 == 0:
                    nc.vector.tensor_copy(stg[:, g, :], quad)
                else:
                    nc.scalar.copy(stg[:, g, :], quad)

        # relayout: stg[32k+h, g, j] -> trib[32k+g, h, j]  (SBUF->SBUF DMA).
        # trib rows are PERMUTED: partition p = 32k+g holds i-row 4g+k.  The
        # host permutes x1nT/mbb columns/rows to match and inverse-permutes
        # the x1u output; o2/l2 sum over i so they are unaffected.
        # compact DRAM bounce in g-quarters so most of it overlaps the
        # x_pair stream; all bounce DMAs on the scalar ring (no contention
        # with the sync-ring x_pair stream)
        trib = sb.tile([IS, H, J], BF16, name="trib", tag="trib")
        scr = drp.tile([4, H, 32, J], BF16, name="scr", tag="scr")
        for q in range(4):
            gs = slice(8 * q, 8 * q + 8)
            for k in range(4):
                nc.scalar.dma_start(scr[k, :, gs, :], stg[32 * k:32 * k + 8, gs, :])
            for k in range(4):
                nc.scalar.dma_start(
                    trib[32 * k + 8 * q:32 * k + 8 * q + 8],
                    scr[k, :, gs, :].rearrange("h g j -> g h j"))
        for h in range(H):
            nc.vector.tensor_add(trib[:, h, :], trib[:, h, :], mbs)

        # ---- mha_1 ----
        o1n = sb.tile([IS, HD], F32, name="o1n", tag="o1n")
        for hg in range(2):
            p1s = [wk.tile([IS, J], BF16, name=f"p1_{hg}_{m}", tag=f"p1{m}",
                           bufs=1) for m in range(4)]
            for blk in range(2):
                bs = slice(blk * 512, (blk + 1) * 512)
                qa = pquad.tile([128, J], F32, name="qa", tag="quad")
                banks = [qa[:, 0:512], qa[:, 512:1024]]
                qb = pquad.tile([128, J], F32, name="qb", tag="quad")
                banks += [qb[:, 0:512], qb[:, 512:1024]]
                for m in range(4):
                    nc.tensor.matmul(banks[m], c_idbf,
                                     trib[:, hg * 4 + m, bs],
                                     start=True, stop=False)
                for m in range(4):
                    nc.tensor.matmul(banks[m], q1T[32 * m:32 * (m + 1), hg, :],
                                     k1T[32 * m:32 * (m + 1), hg, bs],
                                     start=False, stop=True,
                                     tile_position=(32 * m, 0))
                for m in range(4):
                    nc.scalar.activation(p1s[m][:, bs], banks[m], ACTF.Exp)
            for m in range(4):
                h = hg * 4 + m
                p1T = wk.tile([128, 8, IS], BF16, name="p1T", tag="p1T")
                for jt in range(8):
                    tp = ptp.tile([128, 128], BF16, name="tp1", tag="tp")
                    nc.tensor.transpose(tp, p1s[m][:, jt * 128:(jt + 1) * 128],
                                        c_idbf)
                    if jt % 2 == 0:
                        nc.vector.tensor_copy(p1T[:, jt, :], tp)
                    else:
                        nc.scalar.copy(p1T[:, jt, :], tp)
                acc = pmid.tile([IS, D + 1], F32, name="acc1", tag="mid")
                for jt in range(8):
                    nc.tensor.matmul(acc, p1T[:, jt, :], v1a[:, jt, h, :],
                                     start=(jt == 0), stop=(jt == 7))
                r1 = wk.tile([IS, 1], F32, name="r1", tag="r1")
                nc.vector.reciprocal(r1, acc[:, D:D + 1])
                nc.scalar.activation(o1n[:, h * D:(h + 1) * D], acc[:, 0:D],
                                     ACTF.Copy, scale=r1)

        # ---- x1u = x1n + wo1 @ (o1 * g1) + bo1 ----
        og = sb.tile([IS, HD], F32, name="og", tag="og")
        nc.vector.tensor_mul(og, o1n, g1)
        ogT = sb.tile([128, 2, IS], BF16, name="ogT", tag="ogT")
        for t in range(2):
            tp2 = ptp.tile([128, 128], F32, name="tp_og", tag="tp")
            nc.tensor.transpose(tp2, og[:, t * 128:(t + 1) * 128], c_id32)
            nc.vector.tensor_copy(ogT[:, t, :], tp2)
        xop = pmid.tile([C, IS], F32, name="xop", tag="mid")
        for t in range(2):
            nc.tensor.matmul(xop, w("wo1")[:, t * 128:(t + 1) * 128],
                             ogT[:, t, :], start=(t == 0), stop=(t == 1))
        x1uT = sb.tile([C, IS], F32, name="x1uT", tag="x1uT")
        nc.scalar.activation(x1uT, xop, ACTF.Identity, bias=c_bo1)
        nc.vector.tensor_add(x1uT, x1uT, x1nT)
        x1uTb = sb.tile([C, IS], BF16, name="x1uTb", tag="x1uTb")
        nc.vector.tensor_copy(x1uTb, x1uT)

        tpo = ptp.tile([128, 128], F32, name="tp_x1u", tag="tp")
        nc.tensor.transpose(tpo, x1uT, c_id32)
        x1u_sb = sb.tile([IS, C], F32, name="x1u_sb", tag="x1u_sb")
        nc.scalar.copy(x1u_sb, tpo)
        nc.sync.dma_start(x1u_o, x1u_sb)

        # ---- mha_2 projections from x1u ----
        k2T = sb.tile([128, 2, IS], BF16, name="k2T", tag="k2T")
        for hg in range(2):
            kp2 = pmid.tile([128, IS], F32, name="kp2", tag="mid")
            nc.tensor.matmul(kp2, w("wk2")[:, hg * 128:(hg + 1) * 128], x1uTb,
                             start=True, stop=True)
            nc.scalar.copy(k2T[:, hg, :], kp2)
        vp2 = pmid.tile([IS, HD], F32, name="vp2", tag="mid")
        nc.tensor.matmul(vp2, x1uTb, w("wv2"), start=True, stop=True)
        v2sb = sb.tile([IS, HD], BF16, name="v2sb", tag="v2sb")
        nc.vector.tensor_copy(v2sb, vp2)

        # ---- mha_2 partials over local keys ----
        for hg in range(2):
            for blk in range(2):
                bs = slice(blk * 512, (blk + 1) * 512)
                qa = pquad.tile([128, J], F32, name="qa2", tag="quad")
                banks = [qa[:, 0:512], qa[:, 512:1024]]
                qb = pquad.tile([128, J], F32, name="qb2", tag="quad")
                banks += [qb[:, 0:512], qb[:, 512:1024]]
                for m in range(4):
                    nc.tensor.matmul(banks[m], c_idbf,
                                     trib[:, hg * 4 + m, bs],
                                     start=True, stop=False)
                for m in range(4):
                    nc.tensor.matmul(banks[m], k2T[32 * m:32 * (m + 1), hg, :],
                                     q2T[32 * m:32 * (m + 1), hg, bs],
                                     start=False, stop=True,
                                     tile_position=(32 * m, 0))
                p2 = wk.tile([IS, 4, 512], BF16, name="p2", tag="p2")
                for m in range(4):
                    nc.scalar.activation(p2[:, m, :], banks[m], ACTF.Exp)
                av = pmid.tile([128, 512], F32, name="av", tag="mid")
                for m in range(4):
                    h = hg * 4 + m
                    nc.tensor.matmul(av[32 * m:32 * (m + 1), :],
                                     v2sb[:, h * D:(h + 1) * D], p2[:, m, :],
                                     start=True, stop=True,
                                     tile_position=(0, 32 * m))
                l2b = pmid.tile([128, 512], F32, name="l2b", tag="mid")
                for m in range(4):
                    nc.tensor.matmul(l2b[32 * m:32 * m + 1, :], w("ones"),
                                     p2[:, m, :], start=True, stop=True,
                                     tile_position=(0, 32 * m))
                o2sb = wk.tile([128, 512], BF16, name="o2sb", tag="o2sb")
                nc.vector.tensor_copy(o2sb, av)
                nc.sync.dma_start(o2_o[hg, blk], o2sb)
                l2sb = wk.tile([128, 512], BF16, name="l2sb", tag="l2sb")
                nc.scalar.copy(l2sb, l2b)
                l2v = l2sb.rearrange("(m r) j -> m r j", m=4)
                nc.sync.dma_start(l2_o[hg, blk], l2v[:, 0, :])

        for p in reversed((cst, sb, xtp, wk, drp, pquad, pmid, ptp)):
            p.release()

    nc.compile()
    return nc


_CACHE = {}


def _get_program():
    if "nc" not in _CACHE:
        _CACHE["nc"] = build_program()
    return _CACHE["nc"]


def _np_ln(x):
    mu = x.mean(-1, keepdims=True)
    var = np.square(x - mu).mean(-1, keepdims=True)
    return (x - mu) / np.sqrt(var + EPS)


def make_in_maps(x1, x2, x_pair, mask, ln_w, ln_b, wb,
                 wq1, wk1, wv1, wg1, bg1, wo1, bo1,
                 wq2, wk2, wv2, wg2, bg2, wo2, bo2):
    f = np.float32

    def t(a):
        return np.ascontiguousarray(np.asarray(a, f).T)

    lw = np.asarray(ln_w, f)
    lb = np.asarray(ln_b, f)
    x1n = (_np_ln(np.asarray(x1[0], f)) * lw + lb).astype(f)
    x2n = (_np_ln(np.asarray(x2[0], f)) * lw + lb).astype(f)

    wo1t = t(wo1)  # [HD, C]
    blocks = [
        (t(wq1) * ISCALE), t(wk1), t(wv1), t(wg1),
        wo1t.reshape(2, 128, C).transpose(1, 0, 2).reshape(128, 2 * C),
        (t(wq2) * ISCALE), t(wk2), t(wv2),
        np.tile(t(wb), (1, 4)),
        np.eye(128, dtype=f),
        np.ones((128, 1), dtype=f),
    ]
    wcat = np.concatenate(blocks, axis=1).astype(BF)
    assert wcat.shape[1] == NW, (wcat.shape, NW)
    fcat = np.concatenate(
        [np.eye(128, dtype=f), np.asarray(bo1, f)[:, None],
         np.tile(np.asarray(bg1, f), (128, 1))], axis=1)

    shared = {
        "wcat": wcat,
        "fcat": np.ascontiguousarray(fcat),
        "x2nTd": np.ascontiguousarray(x2n.T).astype(BF),
    }
    in_maps = []
    xpnp = np.asarray(x_pair, f)
    msknp = np.asarray(mask, f)
    for m in range(NCORES):
        sl = slice(m * IS, (m + 1) * IS)
        im = dict(shared)
        im["x1nTd"] = np.ascontiguousarray(x1n[sl][PERM].T)
        im["mbb"] = np.ascontiguousarray(
            (INF * (msknp[0, sl] - 1.0))[PERM]).astype(BF)
        im["xpt"] = np.ascontiguousarray(
            xpnp[0, sl].transpose(0, 2, 1)).astype(F8NP)
        in_maps.append(im)
    return in_maps


def combine(results, x2, wg2, bg2, wo2, bo2):
    f = np.float32
    x1u = np.concatenate([results[m]["x1u_o"][IPERM] for m in range(NCORES)],
                         axis=0)[None]
    o2 = np.zeros((H, D, J), dtype=np.float64)
    l2 = np.zeros((H, J), dtype=np.float64)
    for m in range(NCORES):
        o2p = results[m]["o2_o"].astype(np.float64)  # [hg, blk, 128, 512]
        l2p = results[m]["l2_o"].astype(np.float64)  # [hg, blk, 4, 512]
        for hg in range(2):
            for blk in range(2):
                js = slice(blk * 512, (blk + 1) * 512)
                o2[hg * 4:(hg + 1) * 4, :, js] += \
                    o2p[hg, blk].reshape(4, D, 512)
                l2[hg * 4:(hg + 1) * 4, js] += l2p[hg, blk]
    on = (o2 / l2[:, None, :]).astype(f)
    o_fl = on.transpose(2, 0, 1).reshape(J, HD)       # [j, hd]
    x2n = _np_ln(np.asarray(x2[0], f))
    g2 = 1.0 / (1.0 + np.exp(-(x2n @ np.asarray(wg2, f).T
                               + np.asarray(bg2, f))))
    x2u = x2n + (o_fl * g2) @ np.asarray(wo2, f).T + np.asarray(bo2, f)
    return x1u.astype(f), x2u[None].astype(f)


def kernel(**inputs):
    nc = _get_program()
    in_maps = make_in_maps(**inputs)
    res = run_bass_kernel_spmd(nc, in_maps, core_ids=list(range(NCORES)))
    return combine(res.results, inputs["x2"], inputs["wg2"], inputs["bg2"],
                   inputs["wo2"], inputs["bo2"])


if __name__ == "__main__":
    import reference
    inputs = {k: np.asarray(v) for k, v in reference.setup_inputs().items()}
    e1, e2 = reference.reference(**inputs)
    a1, a2 = kernel(**inputs)
    for name, e, a in (("x1u", e1, a1), ("x2u", e2, a2)):
        e = np.asarray(e)
        err = np.abs(a - e).max() / (np.abs(e).max() + 1e-12)
        print(f"{name}: rel_err={err:.3e}")


# revision 22
# speedup vs baseline: 1.4060x; 1.0011x over previous
"""BiDirectionalTriangleAttention on 8 TRN2 NeuronCores (Bass/Tile SPMD).

Sharding: I (row) axis of x1/x_pair/mask split across 8 cores (128 rows each).
Per core:
  - triangle bias tri[h, i_loc, j] from a host-pre-transposed fp8 x_pair shard
    ([i, c, j] layout) so the C contraction lands on SBUF partitions.  4-row
    col-packed matmul quads -> PSUM -> bf16 SBUF staging -> SBUF->SBUF DMA
    relayout into [i_part, h, j].
  - LayerNorm of x1/x2 done on host (host needs x2n for the x2u finish anyway);
    device receives x1nT/x2nT directly.
  - mha_1 fully local (queries = local rows, keys = full x2n).  Scores via
    identity-seeded PSUM (bias) + 4-head row-packed QK matmuls; softmax
    denominators via a ones-column appended to V.
  - mha_2 flash-style partials over the local key rows (keys/values = locally
    updated x1u), 4-head col-packed AV + ones-lhsT exp-sum matmuls.  Host
    merges the 8 partials and applies gating + output projection for x2u.
"""

import numpy as np
import ml_dtypes

import concourse.bass as bass
import concourse.bacc as bacc
import concourse.mybir as mybir
import concourse.tile as tile
from concourse.bass_utils import run_bass_kernel_spmd

F32 = mybir.dt.float32
BF16 = mybir.dt.bfloat16
F8 = mybir.dt.float8e4
BF = ml_dtypes.bfloat16
F8NP = ml_dtypes.float8_e4m3
AX = mybir.AxisListType
ALU = mybir.AluOpType
ACTF = mybir.ActivationFunctionType

B, I, J, C, H, D = 1, 1024, 1024, 128, 8, 32
HD = H * D          # 256
NCORES = 8
IS = I // NCORES    # 128 rows per core
INF = 1e9
EPS = 1e-5
ISCALE = float(1.0 / np.sqrt(np.float32(D)))

GR = 16             # x_pair rows per DMA (2 MiB fp8)
NG = IS // 4        # 32 quad groups of 4 rows

# trib partition p = 32k+g holds local i-row 4g+k
PERM = np.array([4 * (p % 32) + p // 32 for p in range(IS)])   # p -> i
IPERM = np.argsort(PERM)                                       # i -> p

# wcat column layout (bf16)
_WOFF = {}
_off = 0
for _nm, _w in (("wq1", HD), ("wk1", HD), ("wv1", HD), ("wg1", HD),
                ("wo1", HD), ("wq2", HD), ("wk2", HD), ("wv2", HD),
                ("wbt", 4 * H), ("idbf", 128), ("ones", 1)):
    _WOFF[_nm] = (_off, _off + _w)
    _off += _w
NW = _off           # total wcat cols


def build_program():
    nc = bacc.Bacc("TRN2", target_bir_lowering=False, debug=False,
                   num_devices=NCORES)

    def din(name, shape, dt=F32):
        return nc.dram_tensor(name, shape, dt, kind="ExternalInput").ap()

    def dout(name, shape, dt=F32):
        return nc.dram_tensor(name, shape, dt, kind="ExternalOutput").ap()

    xpt = din("xpt", [C, IS, J], F8)       # x_pair shard, [c, i, j] fp8
    wcat = din("wcat", [128, NW], BF16)    # all bf16 weights, concatenated
    fcat = din("fcat", [128, 129 + HD])    # id32 | bo1 | bg1b (f32)
    x2nTd = din("x2nTd", [C, J], BF16)     # LN(x2)^T (host)
    x1nTd = din("x1nTd", [C, IS])          # LN(x1)^T shard (host, f32)
    mbb = din("mbb", [IS, J], BF16)        # INF*(mask-1) shard

    x1u_o = dout("x1u_o", [IS, C])
    o2_o = dout("o2_o", [2, 2, 128, 512], BF16)   # [hg, blk, (m d), j]
    l2_o = dout("l2_o", [2, 2, 4, 512], BF16)     # [hg, blk, m, j]

    with tile.TileContext(nc) as tc:
        cst = tc.alloc_tile_pool(name="cst", bufs=1)
        sb = tc.alloc_tile_pool(name="sb", bufs=1)
        xtp = tc.alloc_tile_pool(name="xtp", bufs=3)
        wk = tc.alloc_tile_pool(name="wk", bufs=2)
        drp = tc.alloc_tile_pool(name="drp", bufs=1, space="DRAM")
        pquad = tc.alloc_tile_pool(name="pquad", bufs=2, space="PSUM")
        pmid = tc.alloc_tile_pool(name="pmid", bufs=2, space="PSUM")
        ptp = tc.alloc_tile_pool(name="ptp", bufs=2, space="PSUM")

        # ---- input DMAs ----
        # consts on the scalar HWDGE ring so they land in parallel with the
        # x_pair stream (sync ring) and the first matmuls start early
        wsb = cst.tile([128, NW], BF16, name="wsb", tag="wsb")
        nc.scalar.dma_start(wsb, wcat)
        fsb = cst.tile([128, 129 + HD], F32, name="fsb", tag="fsb")
        nc.scalar.dma_start(fsb, fcat)
        x2nT = cst.tile([C, J], BF16, name="x2nT", tag="x2nT")
        nc.scalar.dma_start(x2nT, x2nTd)
        x1nT = cst.tile([C, IS], F32, name="x1nT", tag="x1nT")
        nc.scalar.dma_start(x1nT, x1nTd)
        mbs = cst.tile([IS, J], BF16, name="mbs", tag="mbs")
        nc.scalar.dma_start(mbs, mbb)

        xts = []
        for gg in range(8):
            xt = xtp.tile([C, GR, J], F8, name=f"xt{gg}", tag="xt")
            nc.sync.dma_start(xt, xpt[:, gg * GR:(gg + 1) * GR, :])
            xts.append(xt)

        def w(nm):
            lo, hi = _WOFF[nm]
            return wsb[:, lo:hi]

        c_idbf = w("idbf")
        c_id32 = fsb[:, 0:128]
        c_bo1 = fsb[:, 128:129]
        c_bg1b = fsb[:, 129:129 + HD]

        # const ap for activation bias literals
        for cval in (0.0,):
            cap = cst.tile([128, 1], F32, name=f"cap{cval}", tag=f"cap{cval}")
            nc.vector.memset(cap, cval)
            nc.const_aps.aps[(F32, cval)] = cap

        x1nTb = sb.tile([C, IS], BF16, name="x1nTb", tag="x1nTb")
        nc.vector.tensor_copy(x1nTb, x1nT)

        # ---- projections (fill PE while x_pair streams) ----
        # q1T/k1T/q2T/k2T packed: partition 32*(h%4)+d, second index hg=h//4
        q1T = sb.tile([128, 2, IS], BF16, name="q1T", tag="q1T")
        k1T = sb.tile([128, 2, J], BF16, name="k1T", tag="k1T")
        q2T = sb.tile([128, 2, J], BF16, name="q2T", tag="q2T")
        for hg in range(2):
            hs = slice(hg * 128, (hg + 1) * 128)
            qp = pmid.tile([128, IS], F32, name="qp1", tag="mid")
            nc.tensor.matmul(qp, w("wq1")[:, hs], x1nTb, start=True, stop=True)
            nc.scalar.copy(q1T[:, hg, :], qp)
            for blk in range(2):
                bs = slice(blk * 512, (blk + 1) * 512)
                kp = pmid.tile([128, 512], F32, name="kp1", tag="mid")
                nc.tensor.matmul(kp, w("wk1")[:, hs], x2nT[:, bs],
                                 start=True, stop=True)
                if blk == 0:
                    nc.vector.tensor_copy(k1T[:, hg, bs], kp)
                else:
                    nc.scalar.copy(k1T[:, hg, bs], kp)
                qp2 = pmid.tile([128, 512], F32, name="qp2", tag="mid")
                nc.tensor.matmul(qp2, w("wq2")[:, hs], x2nT[:, bs],
                                 start=True, stop=True)
                if blk == 0:
                    nc.scalar.copy(q2T[:, hg, bs], qp2)
                else:
                    nc.vector.tensor_copy(q2T[:, hg, bs], qp2)

        # v1 with ones column appended: [j, jt, h, D+1]
        v1a = sb.tile([128, 8, H, D + 1], BF16, name="v1a", tag="v1a")
        nc.gpsimd.memset(v1a, 1.0)
        for jt in range(8):
            vp = pmid.tile([128, HD], F32, name="vp1", tag="mid")
            nc.tensor.matmul(vp, x2nT[:, jt * 128:(jt + 1) * 128], w("wv1"),
                             start=True, stop=True)
            dst = v1a[:, jt, :, 0:D]
            if jt % 2 == 0:
                nc.vector.tensor_copy(dst, vp.rearrange("p (h d) -> p h d", h=H))
            else:
                nc.scalar.copy(dst, vp.rearrange("p (h d) -> p h d", h=H))

        # gating g1 = sigmoid(x1n @ wg1.T + bg1)
        gp = pmid.tile([IS, HD], F32, name="gp1", tag="mid")
        nc.tensor.matmul(gp, x1nTb, w("wg1"), start=True, stop=True)
        g1 = sb.tile([IS, HD], F32, name="g1", tag="g1")
        nc.vector.tensor_add(g1, gp, c_bg1b)
        nc.scalar.activation(g1, g1, ACTF.Sigmoid)

        # ---- triangle bias: fp8 stream -> quads -> bf16 staging ----
        # relayout stg[32k+h, g, j] -> trib[32k+g, h, j] via a compact DRAM
        # bounce per g-half.  trib rows are PERMUTED: partition p = 32k+g
        # holds i-row 4g+k; host permutes x1nT/mbb and inverse-permutes x1u.
        # Bounce writes ride the idle gpsimd (SWDGE) ring inside the stream;
        # reloads ride the sync ring right after the x_pair stream drains.
        stg = sb.tile([128, NG, J], BF16, name="stg", tag="stg")
        scr = drp.tile([4, H, 32, J], BF16, name="scr", tag="scr")
        for gg in range(8):
            xt = xts[gg]
            for q in range(4):
                g = gg * 4 + q
                quad = pquad.tile([128, J], F32, name="quad", tag="quad")
                for blk in range(2):
                    bs = slice(blk * 512, (blk + 1) * 512)
                    for k in range(4):
                        nc.tensor.matmul(
                            quad[32 * k:32 * (k + 1), bs], w("wbt"),
                            xt[:, q * 4 + k, bs], start=True, stop=True,
                            tile_position=(0, 32 * k))
                if g % 2 == 0:
                    nc.vector.tensor_copy(stg[:, g, :], quad)
                else:
                    nc.scalar.copy(stg[:, g, :], quad)
            if gg in (3, 7):
                gs = slice(0, 16) if gg == 3 else slice(16, 32)
                for k in range(4):
                    nc.gpsimd.dma_start(scr[k, :, gs, :],
                                        stg[32 * k:32 * k + 8, gs, :])

        trib = sb.tile([IS, H, J], BF16, name="trib", tag="trib")
        for half in range(2):
            gs = slice(16 * half, 16 * half + 16)
            for k in range(4):
                nc.sync.dma_start(
                    trib[32 * k + 16 * half:32 * k + 16 * half + 16],
                    scr[k, :, gs, :].rearrange("h g j -> g h j"))
        for h in range(H):
            nc.vector.tensor_add(trib[:, h, :], trib[:, h, :], mbs)

        # ---- mha_1 ----
        o1n = sb.tile([IS, HD], F32, name="o1n", tag="o1n")
        for hg in range(2):
            p1s = [wk.tile([IS, J], BF16, name=f"p1_{hg}_{m}", tag=f"p1{m}",
                           bufs=1) for m in range(4)]
            for blk in range(2):
                bs = slice(blk * 512, (blk + 1) * 512)
                qa = pquad.tile([128, J], F32, name="qa", tag="quad")
                banks = [qa[:, 0:512], qa[:, 512:1024]]
                qb = pquad.tile([128, J], F32, name="qb", tag="quad")
                banks += [qb[:, 0:512], qb[:, 512:1024]]
                for m in range(4):
                    nc.tensor.matmul(banks[m], c_idbf,
                                     trib[:, hg * 4 + m, bs],
                                     start=True, stop=False)
                for m in range(4):
                    nc.tensor.matmul(banks[m], q1T[32 * m:32 * (m + 1), hg, :],
                                     k1T[32 * m:32 * (m + 1), hg, bs],
                                     start=False, stop=True,
                                     tile_position=(32 * m, 0))
                for m in range(4):
                    nc.scalar.activation(p1s[m][:, bs], banks[m], ACTF.Exp)
            for m in range(4):
                h = hg * 4 + m
                p1T = wk.tile([128, 8, IS], BF16, name="p1T", tag="p1T")
                for jt in range(8):
                    tp = ptp.tile([128, 128], BF16, name="tp1", tag="tp")
                    nc.tensor.transpose(tp, p1s[m][:, jt * 128:(jt + 1) * 128],
                                        c_idbf)
                    if jt % 2 == 0:
                        nc.vector.tensor_copy(p1T[:, jt, :], tp)
                    else:
                        nc.scalar.copy(p1T[:, jt, :], tp)
                acc = pmid.tile([IS, D + 1], F32, name="acc1", tag="mid")
                for jt in range(8):
                    nc.tensor.matmul(acc, p1T[:, jt, :], v1a[:, jt, h, :],
                                     start=(jt == 0), stop=(jt == 7))
                r1 = wk.tile([IS, 1], F32, name="r1", tag="r1")
                nc.vector.reciprocal(r1, acc[:, D:D + 1])
                nc.scalar.activation(o1n[:, h * D:(h + 1) * D], acc[:, 0:D],
                                     ACTF.Copy, scale=r1)

        # ---- x1u = x1n + wo1 @ (o1 * g1) + bo1 ----
        og = sb.tile([IS, HD], F32, name="og", tag="og")
        nc.vector.tensor_mul(og, o1n, g1)
        ogT = sb.tile([128, 2, IS], BF16, name="ogT", tag="ogT")
        for t in range(2):
            tp2 = ptp.tile([128, 128], F32, name="tp_og", tag="tp")
            nc.tensor.transpose(tp2, og[:, t * 128:(t + 1) * 128], c_id32)
            nc.vector.tensor_copy(ogT[:, t, :], tp2)
        xop = pmid.tile([C, IS], F32, name="xop", tag="mid")
        for t in range(2):
            nc.tensor.matmul(xop, w("wo1")[:, t * 128:(t + 1) * 128],
                             ogT[:, t, :], start=(t == 0), stop=(t == 1))
        x1uT = sb.tile([C, IS], F32, name="x1uT", tag="x1uT")
        nc.scalar.activation(x1uT, xop, ACTF.Identity, bias=c_bo1)
        nc.vector.tensor_add(x1uT, x1uT, x1nT)
        x1uTb = sb.tile([C, IS], BF16, name="x1uTb", tag="x1uTb")
        nc.vector.tensor_copy(x1uTb, x1uT)

        tpo = ptp.tile([128, 128], F32, name="tp_x1u", tag="tp")
        nc.tensor.transpose(tpo, x1uT, c_id32)
        x1u_sb = sb.tile([IS, C], F32, name="x1u_sb", tag="x1u_sb")
        nc.scalar.copy(x1u_sb, tpo)
        nc.sync.dma_start(x1u_o, x1u_sb)

        # ---- mha_2 projections from x1u ----
        k2T = sb.tile([128, 2, IS], BF16, name="k2T", tag="k2T")
        for hg in range(2):
            kp2 = pmid.tile([128, IS], F32, name="kp2", tag="mid")
            nc.tensor.matmul(kp2, w("wk2")[:, hg * 128:(hg + 1) * 128], x1uTb,
                             start=True, stop=True)
            nc.scalar.copy(k2T[:, hg, :], kp2)
        vp2 = pmid.tile([IS, HD], F32, name="vp2", tag="mid")
        nc.tensor.matmul(vp2, x1uTb, w("wv2"), start=True, stop=True)
        v2sb = sb.tile([IS, HD], BF16, name="v2sb", tag="v2sb")
        nc.vector.tensor_copy(v2sb, vp2)

        # ---- mha_2 partials over local keys ----
        for hg in range(2):
            for blk in range(2):
                bs = slice(blk * 512, (blk + 1) * 512)
                qa = pquad.tile([128, J], F32, name="qa2", tag="quad")
                banks = [qa[:, 0:512], qa[:, 512:1024]]
                qb = pquad.tile([128, J], F32, name="qb2", tag="quad")
                banks += [qb[:, 0:512], qb[:, 512:1024]]
                for m in range(4):
                    nc.tensor.matmul(banks[m], c_idbf,
                                     trib[:, hg * 4 + m, bs],
                                     start=True, stop=False)
                for m in range(4):
                    nc.tensor.matmul(banks[m], k2T[32 * m:32 * (m + 1), hg, :],
                                     q2T[32 * m:32 * (m + 1), hg, bs],
                                     start=False, stop=True,
                                     tile_position=(32 * m, 0))
                p2 = wk.tile([IS, 4, 512], BF16, name="p2", tag="p2")
                for m in range(4):
                    nc.scalar.activation(p2[:, m, :], banks[m], ACTF.Exp)
                av = pmid.tile([128, 512], F32, name="av", tag="mid")
                for m in range(4):
                    h = hg * 4 + m
                    nc.tensor.matmul(av[32 * m:32 * (m + 1), :],
                                     v2sb[:, h * D:(h + 1) * D], p2[:, m, :],
                                     start=True, stop=True,
                                     tile_position=(0, 32 * m))
                l2b = pmid.tile([128, 512], F32, name="l2b", tag="mid")
                for m in range(4):
                    nc.tensor.matmul(l2b[32 * m:32 * m + 1, :], w("ones"),
                                     p2[:, m, :], start=True, stop=True,
                                     tile_position=(0, 32 * m))
                o2sb = wk.tile([128, 512], BF16, name="o2sb", tag="o2sb")
                nc.vector.tensor_copy(o2sb, av)
                nc.sync.dma_start(o2_o[hg, blk], o2sb)
                l2sb = wk.tile([128, 512], BF16, name="l2sb", tag="l2sb")
                nc.scalar.copy(l2sb, l2b)
                l2v = l2sb.rearrange("(m r) j -> m r j", m=4)
                nc.sync.dma_start(l2_o[hg, blk], l2v[:, 0, :])

        for p in reversed((cst, sb, xtp, wk, drp, pquad, pmid, ptp)):
            p.release()

    nc.compile()
    return nc


_CACHE = {}


def _get_program():
    if "nc" not in _CACHE:
        _CACHE["nc"] = build_program()
    return _CACHE["nc"]


def _np_ln(x):
    mu = x.mean(-1, keepdims=True)
    var = np.square(x - mu).mean(-1, keepdims=True)
    return (x - mu) / np.sqrt(var + EPS)


def make_in_maps(x1, x2, x_pair, mask, ln_w, ln_b, wb,
                 wq1, wk1, wv1, wg1, bg1, wo1, bo1,
                 wq2, wk2, wv2, wg2, bg2, wo2, bo2):
    f = np.float32

    def t(a):
        return np.ascontiguousarray(np.asarray(a, f).T)

    lw = np.asarray(ln_w, f)
    lb = np.asarray(ln_b, f)
    x1n = (_np_ln(np.asarray(x1[0], f)) * lw + lb).astype(f)
    x2n = (_np_ln(np.asarray(x2[0], f)) * lw + lb).astype(f)

    wo1t = t(wo1)  # [HD, C]
    blocks = [
        (t(wq1) * ISCALE), t(wk1), t(wv1), t(wg1),
        wo1t.reshape(2, 128, C).transpose(1, 0, 2).reshape(128, 2 * C),
        (t(wq2) * ISCALE), t(wk2), t(wv2),
        np.tile(t(wb), (1, 4)),
        np.eye(128, dtype=f),
        np.ones((128, 1), dtype=f),
    ]
    wcat = np.concatenate(blocks, axis=1).astype(BF)
    assert wcat.shape[1] == NW, (wcat.shape, NW)
    fcat = np.concatenate(
        [np.eye(128, dtype=f), np.asarray(bo1, f)[:, None],
         np.tile(np.asarray(bg1, f), (128, 1))], axis=1)

    shared = {
        "wcat": wcat,
        "fcat": np.ascontiguousarray(fcat),
        "x2nTd": np.ascontiguousarray(x2n.T).astype(BF),
    }
    in_maps = []
    xpnp = np.asarray(x_pair, f)
    msknp = np.asarray(mask, f)
    for m in range(NCORES):
        sl = slice(m * IS, (m + 1) * IS)
        im = dict(shared)
        im["x1nTd"] = np.ascontiguousarray(x1n[sl][PERM].T)
        im["mbb"] = np.ascontiguousarray(
            (INF * (msknp[0, sl] - 1.0))[PERM]).astype(BF)
        im["xpt"] = np.ascontiguousarray(
            xpnp[0, sl].transpose(1, 0, 2)).astype(F8NP)
        in_maps.append(im)
    return in_maps


def combine(results, x2, wg2, bg2, wo2, bo2):
    f = np.float32
    x1u = np.concatenate([results[m]["x1u_o"][IPERM] for m in range(NCORES)],
                         axis=0)[None]
    o2 = np.zeros((H, D, J), dtype=np.float64)
    l2 = np.zeros((H, J), dtype=np.float64)
    for m in range(NCORES):
        o2p = results[m]["o2_o"].astype(np.float64)  # [hg, blk, 128, 512]
        l2p = results[m]["l2_o"].astype(np.float64)  # [hg, blk, 4, 512]
        for hg in range(2):
            for blk in range(2):
                js = slice(blk * 512, (blk + 1) * 512)
                o2[hg * 4:(hg + 1) * 4, :, js] += \
                    o2p[hg, blk].reshape(4, D, 512)
                l2[hg * 4:(hg + 1) * 4, js] += l2p[hg, blk]
    on = (o2 / l2[:, None, :]).astype(f)
    o_fl = on.transpose(2, 0, 1).reshape(J, HD)       # [j, hd]
    x2n = _np_ln(np.asarray(x2[0], f))
    g2 = 1.0 / (1.0 + np.exp(-(x2n @ np.asarray(wg2, f).T
                               + np.asarray(bg2, f))))
    x2u = x2n + (o_fl * g2) @ np.asarray(wo2, f).T + np.asarray(bo2, f)
    return x1u.astype(f), x2u[None].astype(f)


def kernel(**inputs):
    nc = _get_program()
    in_maps = make_in_maps(**inputs)
    res = run_bass_kernel_spmd(nc, in_maps, core_ids=list(range(NCORES)))
    return combine(res.results, inputs["x2"], inputs["wg2"], inputs["bg2"],
                   inputs["wo2"], inputs["bo2"])


if __name__ == "__main__":
    import reference
    inputs = {k: np.asarray(v) for k, v in reference.setup_inputs().items()}
    e1, e2 = reference.reference(**inputs)
    a1, a2 = kernel(**inputs)
    for name, e, a in (("x1u", e1, a1), ("x2u", e2, a2)):
        e = np.asarray(e)
        err = np.abs(a - e).max() / (np.abs(e).max() + 1e-12)
        print(f"{name}: rel_err={err:.3e}")


# revision 24
# speedup vs baseline: 1.4611x; 1.0392x over previous
"""BiDirectionalTriangleAttention on 8 TRN2 NeuronCores (Bass/Tile SPMD).

Sharding: I (row) axis of x1/x_pair/mask split across 8 cores (128 rows each).
Per core:
  - triangle bias tri[h, i_loc, j] from a host-pre-transposed fp8 x_pair shard
    ([i, c, j] layout) so the C contraction lands on SBUF partitions.  4-row
    col-packed matmul quads -> PSUM -> bf16 SBUF staging -> SBUF->SBUF DMA
    relayout into [i_part, h, j].
  - LayerNorm of x1/x2 done on host (host needs x2n for the x2u finish anyway);
    device receives x1nT/x2nT directly.
  - mha_1 fully local (queries = local rows, keys = full x2n).  Scores via
    identity-seeded PSUM (bias) + 4-head row-packed QK matmuls; softmax
    denominators via a ones-column appended to V.
  - mha_2 flash-style partials over the local key rows (keys/values = locally
    updated x1u), 4-head col-packed AV + ones-lhsT exp-sum matmuls.  Host
    merges the 8 partials and applies gating + output projection for x2u.
"""

import numpy as np
import ml_dtypes

import concourse.bass as bass
import concourse.bacc as bacc
import concourse.mybir as mybir
import concourse.tile as tile
from concourse.bass_utils import run_bass_kernel_spmd

F32 = mybir.dt.float32
BF16 = mybir.dt.bfloat16
F8 = mybir.dt.float8e4
BF = ml_dtypes.bfloat16
F8NP = ml_dtypes.float8_e4m3
AX = mybir.AxisListType
ALU = mybir.AluOpType
ACTF = mybir.ActivationFunctionType

B, I, J, C, H, D = 1, 1024, 1024, 128, 8, 32
HD = H * D          # 256
NCORES = 8
IS = I // NCORES    # 128 rows per core
INF = 1e9
EPS = 1e-5
ISCALE = float(1.0 / np.sqrt(np.float32(D)))

GR = 16             # x_pair rows per DMA (2 MiB fp8)
NG = IS // 4        # 32 quad groups of 4 rows

# trib partition p = 32k+g holds local i-row 4g+k
PERM = np.array([4 * (p % 32) + p // 32 for p in range(IS)])   # p -> i
IPERM = np.argsort(PERM)                                       # i -> p

# wcat column layout (bf16)
_WOFF = {}
_off = 0
for _nm, _w in (("wq1", HD), ("wk1", HD), ("wv1", HD), ("wg1", HD),
                ("wo1", HD), ("wq2", HD), ("wk2", HD), ("wv2", HD),
                ("wbt", 4 * H), ("idbf", 128), ("ones", 1)):
    _WOFF[_nm] = (_off, _off + _w)
    _off += _w
NW = _off           # total wcat cols


def build_program():
    nc = bacc.Bacc("TRN2", target_bir_lowering=False, debug=False,
                   num_devices=NCORES)

    def din(name, shape, dt=F32):
        return nc.dram_tensor(name, shape, dt, kind="ExternalInput").ap()

    def dout(name, shape, dt=F32):
        return nc.dram_tensor(name, shape, dt, kind="ExternalOutput").ap()

    xpt = din("xpt", [C, IS, J], F8)       # x_pair shard, [c, i, j] fp8
    wcat = din("wcat", [128, NW], BF16)    # all bf16 weights, concatenated
    fcat = din("fcat", [128, 129 + HD])    # id32 | bo1 | bg1b (f32)
    x2nTd = din("x2nTd", [C, J], BF16)     # LN(x2)^T (host)
    x1nTd = din("x1nTd", [C, IS])          # LN(x1)^T shard (host, f32)
    mbb = din("mbb", [IS, J], BF16)        # INF*(mask-1) shard

    x1u_o = dout("x1u_o", [IS, C])
    o2_o = dout("o2_o", [2, 2, 128, 512], BF16)   # [hg, blk, (m d), j]
    l2_o = dout("l2_o", [2, 2, 4, 512], BF16)     # [hg, blk, m, j]

    with tile.TileContext(nc) as tc:
        cst = tc.alloc_tile_pool(name="cst", bufs=1)
        sb = tc.alloc_tile_pool(name="sb", bufs=1)
        xtp = tc.alloc_tile_pool(name="xtp", bufs=3)
        wk = tc.alloc_tile_pool(name="wk", bufs=2)
        drp = tc.alloc_tile_pool(name="drp", bufs=1, space="DRAM")
        pquad = tc.alloc_tile_pool(name="pquad", bufs=2, space="PSUM")
        pmid = tc.alloc_tile_pool(name="pmid", bufs=2, space="PSUM")
        ptp = tc.alloc_tile_pool(name="ptp", bufs=2, space="PSUM")

        # ---- input DMAs ----
        # matmul-critical consts at the head of the sync ring (in front of
        # the x_pair stream); the rest on the scalar ring
        wsb = cst.tile([128, NW], BF16, name="wsb", tag="wsb")
        nc.sync.dma_start(wsb, wcat)
        x2nT = cst.tile([C, J], BF16, name="x2nT", tag="x2nT")
        nc.sync.dma_start(x2nT, x2nTd)
        x1nT = cst.tile([C, IS], F32, name="x1nT", tag="x1nT")
        nc.sync.dma_start(x1nT, x1nTd)
        fsb = cst.tile([128, 129 + HD], F32, name="fsb", tag="fsb")
        nc.scalar.dma_start(fsb, fcat)
        mbs = cst.tile([IS, J], BF16, name="mbs", tag="mbs")
        nc.scalar.dma_start(mbs, mbb)

        xts = []
        for gg in range(8):
            xt = xtp.tile([C, GR, J], F8, name=f"xt{gg}", tag="xt")
            nc.sync.dma_start(xt, xpt[:, gg * GR:(gg + 1) * GR, :])
            xts.append(xt)

        def w(nm):
            lo, hi = _WOFF[nm]
            return wsb[:, lo:hi]

        c_idbf = w("idbf")
        c_id32 = fsb[:, 0:128]
        c_bo1 = fsb[:, 128:129]
        c_bg1b = fsb[:, 129:129 + HD]

        # const ap for activation bias literals
        for cval in (0.0,):
            cap = cst.tile([128, 1], F32, name=f"cap{cval}", tag=f"cap{cval}")
            nc.vector.memset(cap, cval)
            nc.const_aps.aps[(F32, cval)] = cap

        x1nTb = sb.tile([C, IS], BF16, name="x1nTb", tag="x1nTb")
        nc.vector.tensor_copy(x1nTb, x1nT)

        # ---- projections (fill PE while x_pair streams) ----
        # q1T/k1T/q2T/k2T packed: partition 32*(h%4)+d, second index hg=h//4
        q1T = sb.tile([128, 2, IS], BF16, name="q1T", tag="q1T")
        k1T = sb.tile([128, 2, J], BF16, name="k1T", tag="k1T")
        q2T = sb.tile([128, 2, J], BF16, name="q2T", tag="q2T")
        for hg in range(2):
            hs = slice(hg * 128, (hg + 1) * 128)
            qp = pmid.tile([128, IS], F32, name="qp1", tag="mid")
            nc.tensor.matmul(qp, w("wq1")[:, hs], x1nTb, start=True, stop=True)
            nc.scalar.copy(q1T[:, hg, :], qp)
            for blk in range(2):
                bs = slice(blk * 512, (blk + 1) * 512)
                kp = pmid.tile([128, 512], F32, name="kp1", tag="mid")
                nc.tensor.matmul(kp, w("wk1")[:, hs], x2nT[:, bs],
                                 start=True, stop=True)
                if blk == 0:
                    nc.vector.tensor_copy(k1T[:, hg, bs], kp)
                else:
                    nc.scalar.copy(k1T[:, hg, bs], kp)
                qp2 = pmid.tile([128, 512], F32, name="qp2", tag="mid")
                nc.tensor.matmul(qp2, w("wq2")[:, hs], x2nT[:, bs],
                                 start=True, stop=True)
                if blk == 0:
                    nc.scalar.copy(q2T[:, hg, bs], qp2)
                else:
                    nc.vector.tensor_copy(q2T[:, hg, bs], qp2)

        # v1 with ones column appended: [j, jt, h, D+1]
        v1a = sb.tile([128, 8, H, D + 1], BF16, name="v1a", tag="v1a")
        nc.gpsimd.memset(v1a, 1.0)
        for jt in range(8):
            vp = pmid.tile([128, HD], F32, name="vp1", tag="mid")
            nc.tensor.matmul(vp, x2nT[:, jt * 128:(jt + 1) * 128], w("wv1"),
                             start=True, stop=True)
            dst = v1a[:, jt, :, 0:D]
            if jt % 2 == 0:
                nc.vector.tensor_copy(dst, vp.rearrange("p (h d) -> p h d", h=H))
            else:
                nc.scalar.copy(dst, vp.rearrange("p (h d) -> p h d", h=H))

        # gating g1 = sigmoid(x1n @ wg1.T + bg1)
        gp = pmid.tile([IS, HD], F32, name="gp1", tag="mid")
        nc.tensor.matmul(gp, x1nTb, w("wg1"), start=True, stop=True)
        g1 = sb.tile([IS, HD], F32, name="g1", tag="g1")
        nc.vector.tensor_add(g1, gp, c_bg1b)
        nc.scalar.activation(g1, g1, ACTF.Sigmoid)

        # ---- triangle bias: fp8 stream -> quads -> bf16 staging ----
        # relayout stg[32k+h, g, j] -> trib[32k+g, h, j] via a compact DRAM
        # bounce per g-half.  trib rows are PERMUTED: partition p = 32k+g
        # holds i-row 4g+k; host permutes x1nT/mbb and inverse-permutes x1u.
        # Bounce writes ride the idle gpsimd (SWDGE) ring inside the stream;
        # reloads ride the sync ring right after the x_pair stream drains.
        stg = sb.tile([128, NG, J], BF16, name="stg", tag="stg")
        scr = drp.tile([4, H, 32, J], BF16, name="scr", tag="scr")
        for gg in range(8):
            xt = xts[gg]
            for q in range(4):
                g = gg * 4 + q
                quad = pquad.tile([128, J], F32, name="quad", tag="quad")
                # warm-keeper dummies: run while the next x_pair chunk is in
                # flight (no xt dep), keeping HAM at 2.4 GHz; real matmuls
                # overwrite via start=True
                for dmy in range(3):
                    nc.tensor.matmul(quad[:, 512 * (dmy % 2):512 * (dmy % 2) + 512],
                                     c_idbf, wsb[:, 0:512], start=True,
                                     stop=True, skip_group_check=True)
                for blk in range(2):
                    bs = slice(blk * 512, (blk + 1) * 512)
                    for k in range(4):
                        nc.tensor.matmul(
                            quad[32 * k:32 * (k + 1), bs], w("wbt"),
                            xt[:, q * 4 + k, bs], start=True, stop=True,
                            tile_position=(0, 32 * k), skip_group_check=True)
                if g % 2 == 0:
                    nc.vector.tensor_copy(stg[:, g, :], quad)
                else:
                    nc.scalar.copy(stg[:, g, :], quad)
            if gg in (3, 7):
                gs = slice(0, 16) if gg == 3 else slice(16, 32)
                for k in range(4):
                    nc.scalar.dma_start(scr[k, :, gs, :],
                                        stg[32 * k:32 * k + 8, gs, :])

        trib = sb.tile([IS, H, J], BF16, name="trib", tag="trib")
        for half in range(2):
            gs = slice(16 * half, 16 * half + 16)
            for k in range(4):
                nc.sync.dma_start(
                    trib[32 * k + 16 * half:32 * k + 16 * half + 16],
                    scr[k, :, gs, :].rearrange("h g j -> g h j"))
        for h in range(H):
            nc.vector.tensor_add(trib[:, h, :], trib[:, h, :], mbs)

        # ---- mha_1 ----
        o1n = sb.tile([IS, HD], F32, name="o1n", tag="o1n")
        for hg in range(2):
            p1s = [wk.tile([IS, J], BF16, name=f"p1_{hg}_{m}", tag=f"p1{m}",
                           bufs=1) for m in range(4)]
            for blk in range(2):
                bs = slice(blk * 512, (blk + 1) * 512)
                qa = pquad.tile([128, J], F32, name="qa", tag="quad")
                banks = [qa[:, 0:512], qa[:, 512:1024]]
                qb = pquad.tile([128, J], F32, name="qb", tag="quad")
                banks += [qb[:, 0:512], qb[:, 512:1024]]
                for m in range(4):
                    nc.tensor.matmul(banks[m], c_idbf,
                                     trib[:, hg * 4 + m, bs],
                                     start=True, stop=False)
                for m in range(4):
                    nc.tensor.matmul(banks[m], q1T[32 * m:32 * (m + 1), hg, :],
                                     k1T[32 * m:32 * (m + 1), hg, bs],
                                     start=False, stop=True,
                                     tile_position=(32 * m, 0))
                for m in range(4):
                    nc.scalar.activation(p1s[m][:, bs], banks[m], ACTF.Exp)
            for m in range(4):
                h = hg * 4 + m
                p1T = wk.tile([128, 8, IS], BF16, name="p1T", tag="p1T")
                for jt in range(8):
                    tp = ptp.tile([128, 128], BF16, name="tp1", tag="tp")
                    nc.tensor.transpose(tp, p1s[m][:, jt * 128:(jt + 1) * 128],
                                        c_idbf)
                    if jt % 2 == 0:
                        nc.vector.tensor_copy(p1T[:, jt, :], tp)
                    else:
                        nc.scalar.copy(p1T[:, jt, :], tp)
                acc = pmid.tile([IS, D + 1], F32, name="acc1", tag="mid")
                for jt in range(8):
                    nc.tensor.matmul(acc, p1T[:, jt, :], v1a[:, jt, h, :],
                                     start=(jt == 0), stop=(jt == 7))
                r1 = wk.tile([IS, 1], F32, name="r1", tag="r1")
                nc.vector.reciprocal(r1, acc[:, D:D + 1])
                nc.scalar.activation(o1n[:, h * D:(h + 1) * D], acc[:, 0:D],
                                     ACTF.Copy, scale=r1)

        # ---- x1u = x1n + wo1 @ (o1 * g1) + bo1 ----
        og = sb.tile([IS, HD], F32, name="og", tag="og")
        nc.vector.tensor_mul(og, o1n, g1)
        ogT = sb.tile([128, 2, IS], BF16, name="ogT", tag="ogT")
        for t in range(2):
            tp2 = ptp.tile([128, 128], F32, name="tp_og", tag="tp")
            nc.tensor.transpose(tp2, og[:, t * 128:(t + 1) * 128], c_id32)
            nc.vector.tensor_copy(ogT[:, t, :], tp2)
        xop = pmid.tile([C, IS], F32, name="xop", tag="mid")
        for t in range(2):
            nc.tensor.matmul(xop, w("wo1")[:, t * 128:(t + 1) * 128],
                             ogT[:, t, :], start=(t == 0), stop=(t == 1))
        x1uT = sb.tile([C, IS], F32, name="x1uT", tag="x1uT")
        nc.scalar.activation(x1uT, xop, ACTF.Identity, bias=c_bo1)
        nc.vector.tensor_add(x1uT, x1uT, x1nT)
        x1uTb = sb.tile([C, IS], BF16, name="x1uTb", tag="x1uTb")
        nc.vector.tensor_copy(x1uTb, x1uT)

        tpo = ptp.tile([128, 128], F32, name="tp_x1u", tag="tp")
        nc.tensor.transpose(tpo, x1uT, c_id32)
        x1u_sb = sb.tile([IS, C], F32, name="x1u_sb", tag="x1u_sb")
        nc.scalar.copy(x1u_sb, tpo)
        nc.sync.dma_start(x1u_o, x1u_sb)

        # ---- mha_2 projections from x1u ----
        k2T = sb.tile([128, 2, IS], BF16, name="k2T", tag="k2T")
        for hg in range(2):
            kp2 = pmid.tile([128, IS], F32, name="kp2", tag="mid")
            nc.tensor.matmul(kp2, w("wk2")[:, hg * 128:(hg + 1) * 128], x1uTb,
                             start=True, stop=True)
            nc.scalar.copy(k2T[:, hg, :], kp2)
        vp2 = pmid.tile([IS, HD], F32, name="vp2", tag="mid")
        nc.tensor.matmul(vp2, x1uTb, w("wv2"), start=True, stop=True)
        v2sb = sb.tile([IS, HD], BF16, name="v2sb", tag="v2sb")
        nc.vector.tensor_copy(v2sb, vp2)

        # ---- mha_2 partials over local keys ----
        for hg in range(2):
            for blk in range(2):
                bs = slice(blk * 512, (blk + 1) * 512)
                qa = pquad.tile([128, J], F32, name="qa2", tag="quad")
                banks = [qa[:, 0:512], qa[:, 512:1024]]
                qb = pquad.tile([128, J], F32, name="qb2", tag="quad")
                banks += [qb[:, 0:512], qb[:, 512:1024]]
                for m in range(4):
                    nc.tensor.matmul(banks[m], c_idbf,
                                     trib[:, hg * 4 + m, bs],
                                     start=True, stop=False)
                for m in range(4):
                    nc.tensor.matmul(banks[m], k2T[32 * m:32 * (m + 1), hg, :],
                                     q2T[32 * m:32 * (m + 1), hg, bs],
                                     start=False, stop=True,
                                     tile_position=(32 * m, 0))
                p2 = wk.tile([IS, 4, 512], BF16, name="p2", tag="p2")
                for m in range(4):
                    nc.scalar.activation(p2[:, m, :], banks[m], ACTF.Exp)
                av = pmid.tile([128, 512], F32, name="av", tag="mid")
                for m in range(4):
                    h = hg * 4 + m
                    nc.tensor.matmul(av[32 * m:32 * (m + 1), :],
                                     v2sb[:, h * D:(h + 1) * D], p2[:, m, :],
                                     start=True, stop=True,
                                     tile_position=(0, 32 * m))
                l2b = pmid.tile([128, 512], F32, name="l2b", tag="mid")
                for m in range(4):
                    nc.tensor.matmul(l2b[32 * m:32 * m + 1, :], w("ones"),
                                     p2[:, m, :], start=True, stop=True,
                                     tile_position=(0, 32 * m))
                o2sb = wk.tile([128, 512], BF16, name="o2sb", tag="o2sb")
                nc.vector.tensor_copy(o2sb, av)
                nc.sync.dma_start(o2_o[hg, blk], o2sb)
                l2sb = wk.tile([128, 512], BF16, name="l2sb", tag="l2sb")
                nc.scalar.copy(l2sb, l2b)
                l2v = l2sb.rearrange("(m r) j -> m r j", m=4)
                nc.sync.dma_start(l2_o[hg, blk], l2v[:, 0, :])

        for p in reversed((cst, sb, xtp, wk, drp, pquad, pmid, ptp)):
            p.release()

    nc.compile()
    return nc


_CACHE = {}


def _get_program():
    if "nc" not in _CACHE:
        _CACHE["nc"] = build_program()
    return _CACHE["nc"]


def _np_ln(x):
    mu = x.mean(-1, keepdims=True)
    var = np.square(x - mu).mean(-1, keepdims=True)
    return (x - mu) / np.sqrt(var + EPS)


def make_in_maps(x1, x2, x_pair, mask, ln_w, ln_b, wb,
                 wq1, wk1, wv1, wg1, bg1, wo1, bo1,
                 wq2, wk2, wv2, wg2, bg2, wo2, bo2):
    f = np.float32

    def t(a):
        return np.ascontiguousarray(np.asarray(a, f).T)

    lw = np.asarray(ln_w, f)
    lb = np.asarray(ln_b, f)
    x1n = (_np_ln(np.asarray(x1[0], f)) * lw + lb).astype(f)
    x2n = (_np_ln(np.asarray(x2[0], f)) * lw + lb).astype(f)

    wo1t = t(wo1)  # [HD, C]
    blocks = [
        (t(wq1) * ISCALE), t(wk1), t(wv1), t(wg1),
        wo1t.reshape(2, 128, C).transpose(1, 0, 2).reshape(128, 2 * C),
        (t(wq2) * ISCALE), t(wk2), t(wv2),
        np.tile(t(wb), (1, 4)),
        np.eye(128, dtype=f),
        np.ones((128, 1), dtype=f),
    ]
    wcat = np.concatenate(blocks, axis=1).astype(BF)
    assert wcat.shape[1] == NW, (wcat.shape, NW)
    fcat = np.concatenate(
        [np.eye(128, dtype=f), np.asarray(bo1, f)[:, None],
         np.tile(np.asarray(bg1, f), (128, 1))], axis=1)

    shared = {
        "wcat": wcat,
        "fcat": np.ascontiguousarray(fcat),
        "x2nTd": np.ascontiguousarray(x2n.T).astype(BF),
    }
    in_maps = []
    xpnp = np.asarray(x_pair, f)
    msknp = np.asarray(mask, f)
    for m in range(NCORES):
        sl = slice(m * IS, (m + 1) * IS)
        im = dict(shared)
        im["x1nTd"] = np.ascontiguousarray(x1n[sl][PERM].T)
        im["mbb"] = np.ascontiguousarray(
            (INF * (msknp[0, sl] - 1.0))[PERM]).astype(BF)
        im["xpt"] = np.ascontiguousarray(
            xpnp[0, sl].transpose(1, 0, 2)).astype(F8NP)
        in_maps.append(im)
    return in_maps


def combine(results, x2, wg2, bg2, wo2, bo2):
    f = np.float32
    x1u = np.concatenate([results[m]["x1u_o"][IPERM] for m in range(NCORES)],
                         axis=0)[None]
    o2 = np.zeros((H, D, J), dtype=np.float64)
    l2 = np.zeros((H, J), dtype=np.float64)
    for m in range(NCORES):
        o2p = results[m]["o2_o"].astype(np.float64)  # [hg, blk, 128, 512]
        l2p = results[m]["l2_o"].astype(np.float64)  # [hg, blk, 4, 512]
        for hg in range(2):
            for blk in range(2):
                js = slice(blk * 512, (blk + 1) * 512)
                o2[hg * 4:(hg + 1) * 4, :, js] += \
                    o2p[hg, blk].reshape(4, D, 512)
                l2[hg * 4:(hg + 1) * 4, js] += l2p[hg, blk]
    on = (o2 / l2[:, None, :]).astype(f)
    o_fl = on.transpose(2, 0, 1).reshape(J, HD)       # [j, hd]
    x2n = _np_ln(np.asarray(x2[0], f))
    g2 = 1.0 / (1.0 + np.exp(-(x2n @ np.asarray(wg2, f).T
                               + np.asarray(bg2, f))))
    x2u = x2n + (o_fl * g2) @ np.asarray(wo2, f).T + np.asarray(bo2, f)
    return x1u.astype(f), x2u[None].astype(f)


def kernel(**inputs):
    nc = _get_program()
    in_maps = make_in_maps(**inputs)
    res = run_bass_kernel_spmd(nc, in_maps, core_ids=list(range(NCORES)))
    return combine(res.results, inputs["x2"], inputs["wg2"], inputs["bg2"],
                   inputs["wo2"], inputs["bo2"])


if __name__ == "__main__":
    import reference
    inputs = {k: np.asarray(v) for k, v in reference.setup_inputs().items()}
    e1, e2 = reference.reference(**inputs)
    a1, a2 = kernel(**inputs)
    for name, e, a in (("x1u", e1, a1), ("x2u", e2, a2)):
        e = np.asarray(e)
        err = np.abs(a - e).max() / (np.abs(e).max() + 1e-12)
        print(f"{name}: rel_err={err:.3e}")
